# revision 1
# baseline (speedup 1.0000x reference)
"""Trainium2 Bass kernel for nn_CosSimRouter_learn_49778670960796.

Host: cosine-similarity scoring / sort / gather (tiny, shape-determining).
Device (8 NeuronCores, tensor-parallel over heads/hidden):
  3x MHA + FFN + logits. fp16 weights/activations (halves HBM traffic vs
  fp32; ~2e-4 rounding is far below the ~1e-2 top-k selection margin),
  fp32 PSUM accumulation and LN/softmax statistics. Exact token counts
  (no 128-padding of the token free dim). Host-packed weight layouts so
  each weight streams in as a few large DMAs. Collectives: fp16 ARs for
  the three residual streams, fp16 ReduceScatter for the final-LN s2
  stat, and one tiny fp32 AR carrying the linear stats (s1 via
  host-precomputed colsum(Wf2), Ws-dot via host-precomputed Wf2^T Ws).
Host: top-k + final gather (exact rows of the input).
"""

import numpy as np

E = 4096
H = 16
HID = 8192
GAMMA = 0.2
TEMP = 0.05
EXPAND = 0.7
NCORES = 8
ET = E // 128  # 32 feature tiles
DH = E // H  # 256
HL = H // NCORES  # 2 heads per core
DLOC = HL * DH  # 512 local head dims
FLOC = HID // NCORES  # 1024 local ffn hidden
KG = 8  # k-blocks per weight/act chunk

_CACHE = {}


# ----------------------------------------------------------------------------
# host-side reference math (numpy, fp32) for the scoring stage + fallback
# ----------------------------------------------------------------------------

def _score_partition(vision_feature, text_embed, attention_mask):
    vf = vision_feature.astype(np.float32)
    te = text_embed.astype(np.float32)
    vn = vf / np.maximum(np.linalg.norm(vf, axis=-1, keepdims=True), 1e-8)
    tn = te / np.maximum(np.linalg.norm(te, axis=-1, keepdims=True), 1e-8)
    cs = vn @ tn.T
    cs = np.where(attention_mask[None, :], cs, np.float32(0.0))
    m = cs.max(axis=-1) / np.float32(TEMP)
    e = np.exp(m - m.max())
    scores = e / e.sum()
    order = np.argsort(-scores, kind="stable")
    cum = np.cumsum(scores[order])
    t = int((cum <= GAMMA).sum())
    return t, order[:t], order[t:]


def _ln_np(x):
    m = x.mean(-1, keepdims=True)
    v = ((x - m) ** 2).mean(-1, keepdims=True)
    return (x - m) / np.sqrt(v + 1e-5)


def _gelu_np(x):
    import math

    erf = np.frompyfunc(math.erf, 1, 1)
    return (x * 0.5 * (1.0 + erf(x / math.sqrt(2.0)).astype(np.float64))
            ).astype(x.dtype)


def _mha_np(q_in, kv_in, Wqkv, bqkv, Wo, bo):
    dh = E // H
    Wq, Wk, Wv = np.split(Wqkv, 3, axis=0)
    bq, bk, bv = np.split(bqkv, 3)
    q = (q_in @ Wq.T + bq).reshape(-1, H, dh)
    k = (kv_in @ Wk.T + bk).reshape(-1, H, dh)
    v = (kv_in @ Wv.T + bv).reshape(-1, H, dh)
    att = np.einsum("qhd,khd->hqk", q, k) / np.float32(np.sqrt(dh))
    att = att - att.max(-1, keepdims=True)
    att = np.exp(att)
    att /= att.sum(-1, keepdims=True)
    o = np.einsum("hqk,khd->qhd", att.astype(np.float32), v).reshape(-1, E)
    return o @ Wo.T + bo


def _reference_np(vision_feature, text_embed, attention_mask,
                  Wqkv1, bqkv1, Wo1, bo1, Wqkv2, bqkv2, Wo2, bo2,
                  Wqkvc, bqkvc, Woc, boc, Wf1, bf1, Wf2, bf2, Ws, bs):
    t, sel_idx, rem_idx = _score_partition(vision_feature, text_embed,
                                           attention_mask)
    sel = vision_feature[sel_idx]
    rem = vision_feature[rem_idx]
    cat = np.concatenate([sel, text_embed], axis=0)
    x = _ln_np(_mha_np(cat, cat, Wqkv1, bqkv1, Wo1, bo1) + cat)
    r = _ln_np(_mha_np(rem, rem, Wqkv2, bqkv2, Wo2, bo2) + rem)
    x = _ln_np(_mha_np(r, x, Wqkvc, bqkvc, Woc, boc) + r)
    ffn = _gelu_np(x @ Wf1.T + bf1) @ Wf2.T + bf2
    x = _ln_np(x + ffn)
    logits = (x @ Ws.T + bs).squeeze(-1)
    es = 1.0 / (1.0 + np.exp(-logits))
    k = int(t * EXPAND)
    ei = np.argsort(-es, kind="stable")[:k]
    final = np.sort(np.concatenate([sel_idx, rem_idx[ei]]))
    return vision_feature[final]


# ----------------------------------------------------------------------------
# device program
# ----------------------------------------------------------------------------

def _build_device(ncat, nrem, dumps=False):
    import concourse.bacc as bacc
    import concourse.mybir as mybir
    import concourse.tile as tile

    dt = mybir.dt
    F32 = dt.float32
    F16 = dt.float16
    AF = mybir.ActivationFunctionType
    ALU = mybir.AluOpType

    JC = (ncat + 127) // 128  # kv partition tiles for cat (2)
    JR = (nrem + 127) // 128  # kv partition tiles for rem (4)

    nc = bacc.Bacc("TRN2", target_bir_lowering=False, debug=False,
                   num_devices=NCORES)

    # ---------------- DRAM I/O (all host-packed, see _prep_in_maps) --------
    remp_d = nc.dram_tensor("remp", [128, ET * nrem], F16, kind="ExternalInput")
    catp_d = nc.dram_tensor("catp", [128, ET * ncat], F16, kind="ExternalInput")
    wd = {}
    for l in ("1", "2", "c"):
        for p in ("q", "k", "v"):
            wd[p + l] = nc.dram_tensor(f"w{p}{l}", [128, ET * DLOC], F16,
                                       kind="ExternalInput")
        wd["o" + l] = nc.dram_tensor(f"wo{l}", [128, (DLOC // 128) * E], F16,
                                     kind="ExternalInput")
    wd["f1"] = nc.dram_tensor("wf1", [128, ET * FLOC], F16,
                              kind="ExternalInput")
    wd["f2"] = nc.dram_tensor("wf2", [128, (FLOC // 128) * E], F16,
                              kind="ExternalInput")
    ws_d = nc.dram_tensor("wsp", [128, ET], F16, kind="ExternalInput")
    c2w_d = nc.dram_tensor("c2w", [128, 2 * (FLOC // 128)], F16,
                           kind="ExternalInput")
    f1rs_d = nc.dram_tensor("f1rs", [1, FLOC], F16, kind="ExternalInput")
    consts_d = nc.dram_tensor("consts", [128, 2], F32, kind="ExternalInput")
    logits_d = nc.dram_tensor("logits", [1, 512], F32, kind="ExternalOutput")
    dbg = {}
    if dumps:
        for nm, L in (("dbg_x1", ncat), ("dbg_r", nrem), ("dbg_x2", nrem)):
            dbg[nm] = nc.dram_tensor(nm, [128, ET * L], F16,
                                     kind="ExternalOutput")

    replica = [list(range(NCORES))]
    NG_R = ET // KG  # 4 act groups for rem
    NG_C = ET // KG  # 4 act groups for cat

    with tile.TileContext(nc, num_cores=NCORES) as tc:
        with (
            tc.tile_pool(name="acts", bufs=1) as acts,
            tc.tile_pool(name="psum", bufs=1, space="PSUM") as psum,
            tc.tile_pool(name="dram", bufs=1, space="DRAM") as dram,
        ):
            # ---- constants ----
            ones_col = acts.tile([128, 1], F16, name="ones_col",
                                 tag="ones_col")
            nc.vector.memset(ones_col[:], 1.0)
            ones_row = acts.tile([1, 128], F16, name="ones_row",
                                 tag="ones_row")
            nc.vector.memset(ones_row[:], 1.0)
            ws_sb = acts.tile([128, ET], F16, name="ws_sb", tag="ws_sb")
            nc.sync.dma_start(ws_sb[:], ws_d.ap())
            c2w_sb = acts.tile([128, 2 * (FLOC // 128)], F16, name="c2w_sb",
                               tag="c2w_sb")
            nc.sync.dma_start(c2w_sb[:], c2w_d.ap())
            consts = acts.tile([128, 2], F32, name="consts", tag="consts")
            nc.sync.dma_start(consts[:], consts_d.ap())
            f1rs_sb = acts.tile([1, FLOC], F16, name="f1rs_sb", tag="f1rs")
            nc.sync.dma_start(f1rs_sb[:], f1rs_d.ap())

            def pp(name, L, parts=128):
                t_ = psum.tile([128, L], F32, name=name, tag="pp", bufs=8)
                return t_[0:parts, :] if parts < 128 else t_[:]

            def pstat(name, L):
                return psum.tile([1, L], F32, name=name, tag="pp", bufs=8)[:]

            def wchunk(name, cols):
                return acts.tile([128, cols], F16, name=name, tag="wt",
                                 bufs=3, padded_shape=[128, KG * FLOC])

            # ---- activations: group tiles + slice helper ----
            def load_x(name, dram_t, L, ngroups):
                ts = []
                for g in range(ngroups):
                    xt = acts.tile([128, KG * L], F16, name=f"{name}_{g}",
                                   tag=f"{name}_{g}")
                    nc.sync.dma_start(
                        xt[:], dram_t.ap()[:, KG * L * g:KG * L * (g + 1)])
                    ts.append(xt)
                return ts

            def xs(ts, L, k):
                g, kk = k // KG, k % KG
                return ts[g][:, kk * L:(kk + 1) * L]

            # ---------------- building blocks ----------------
            def proj_fm(tagbase, w_dram, x_ts, L, outtag):
                """q/k projection -> 4 tiles [128, L] fp16 (DLOC, L) layout."""
                chunks = []
                for g in range(ET // KG):
                    wt = wchunk(f"w_{tagbase}_{g}", KG * DLOC)
                    nc.sync.dma_start(
                        wt[:],
                        w_dram.ap()[:, KG * DLOC * g:KG * DLOC * (g + 1)])
                    chunks.append(wt)
                ps = [pp(f"ps_{tagbase}_{m}", L) for m in range(4)]
                for k in range(ET):
                    g, kk = k // KG, k % KG
                    for m in range(4):
                        nc.tensor.matmul(
                            ps[m],
                            chunks[g][:, kk * DLOC + 128 * m:
                                      kk * DLOC + 128 * (m + 1)],
                            xs(x_ts, L, k),
                            start=(k == 0), stop=(k == ET - 1))
                outs = []
                for m in range(4):
                    o = acts.tile([128, L], F16, name=f"{tagbase}_{m}",
                                  tag=f"{outtag}_{m}")
                    nc.scalar.copy(o[:], ps[m])
                    outs.append(o)
                return outs

            def proj_tm(tagbase, w_dram, x_ts, L, JT):
                """v projection -> JT tiles [128, DLOC] fp16 (kv, DLOC)."""
                chunks = []
                for g in range(ET // KG):
                    wt = wchunk(f"w_{tagbase}_{g}", KG * DLOC)
                    nc.sync.dma_start(
                        wt[:],
                        w_dram.ap()[:, KG * DLOC * g:KG * DLOC * (g + 1)])
                    chunks.append(wt)
                ps = []
                for j in range(JT):
                    pj = min(128, L - 128 * j)
                    ps.append(pp(f"ps_{tagbase}_{j}", DLOC, parts=pj))
                for k in range(ET):
                    g, kk = k // KG, k % KG
                    for j in range(JT):
                        pj = min(128, L - 128 * j)
                        nc.tensor.matmul(
                            ps[j],
                            xs(x_ts, L, k)[:, 128 * j:128 * j + pj],
                            chunks[g][:, kk * DLOC:(kk + 1) * DLOC],
                            start=(k == 0), stop=(k == ET - 1))
                outs = []
                for j in range(JT):
                    pj = min(128, L - 128 * j)
                    o = acts.tile([128, DLOC], F16, name=f"{tagbase}_{j}",
                                  tag=f"v_{j}")
                    nc.scalar.copy(o[0:pj, :], ps[j])
                    outs.append(o)
                return outs

            def attention(tag, qT, kT, vT, Lq, Lkv, JT):
                # Emission order keeps the PE queue free of stalls: all
                # score/dsum/po matmuls are independent of the softmax
                # denominator chain (DVE), which runs concurrently; the
                # rrep broadcast matmuls come last.
                exps_h = []
                for h in range(HL):
                    exps = []
                    for j in range(JT):
                        pj = min(128, Lkv - 128 * j)
                        p = pp(f"ps_s_{tag}_{h}_{j}", Lq, parts=pj)
                        for c in range(2):
                            nc.tensor.matmul(
                                p,
                                kT[2 * h + c][:, 128 * j:128 * j + pj],
                                qT[2 * h + c][:],
                                start=(c == 0), stop=(c == 1))
                        e = acts.tile([128, Lq], F16,
                                      name=f"es_{tag}_{h}_{j}",
                                      tag=f"expS_{h}_{j}")
                        nc.scalar.activation(e[0:pj, :], p, AF.Exp,
                                             scale=float(1.0 / np.sqrt(DH)))
                        exps.append(e)
                    exps_h.append(exps)
                rec2s = []
                for h in range(HL):
                    dsum = pstat(f"ps_d_{tag}_{h}", Lq)
                    for j in range(JT):
                        pj = min(128, Lkv - 128 * j)
                        nc.tensor.matmul(dsum, ones_col[0:pj, :],
                                         exps_h[h][j][0:pj, :],
                                         start=(j == 0), stop=(j == JT - 1))
                    den = acts.tile([1, Lq], F32, name=f"den_{tag}_{h}",
                                    tag="aden")
                    rec = acts.tile([1, Lq], F32, name=f"rec_{tag}_{h}",
                                    tag="arec")
                    nc.vector.tensor_copy(den[:], dsum)
                    nc.vector.reciprocal(rec[:], den[:])
                    nc.vector.tensor_tensor(den[:], den[:], rec[:], ALU.mult)
                    nc.vector.tensor_scalar(den[:], den[:], -1.0, 2.0,
                                            ALU.mult, ALU.add)
                    rec2 = acts.tile([1, Lq], F16, name=f"rec2_{tag}_{h}",
                                     tag=f"rec2_{h}")
                    nc.vector.tensor_tensor(rec2[:], rec[:], den[:], ALU.mult)
                    rec2s.append(rec2)
                pos = []
                for h in range(HL):
                    for c in range(2):
                        po = pp(f"ps_o_{tag}_{h}_{c}", Lq)
                        for j in range(JT):
                            pj = min(128, Lkv - 128 * j)
                            nc.tensor.matmul(
                                po,
                                vT[j][0:pj, 256 * h + 128 * c:
                                      256 * h + 128 * (c + 1)],
                                exps_h[h][j][0:pj, :],
                                start=(j == 0), stop=(j == JT - 1))
                        pos.append(po)
                oT = []
                for h in range(HL):
                    rrep_p = pp(f"ps_rr_{tag}_{h}", Lq)
                    nc.tensor.matmul(rrep_p, ones_row[:], rec2s[h][:],
                                     start=True, stop=True)
                    rrep = acts.tile([128, Lq], F32, name=f"rr_{tag}_{h}",
                                     tag=f"rrep_{h}")
                    nc.scalar.copy(rrep[:], rrep_p)
                    for c in range(2):
                        o = acts.tile([128, Lq], F16,
                                      name=f"oT_{tag}_{h}_{c}",
                                      tag=f"oT_{2 * h + c}")
                        nc.vector.tensor_tensor(o[:], pos[2 * h + c],
                                                rrep[:], ALU.mult)
                        oT.append(o)
                return oT

            def out_proj_to_dram(tag, oT, w_dram, ar_in_halves,
                                 ar_out_halves, Lq):
                """4 quarter chunks; psum group of 8 m-tiles per quarter;
                AllReduce fired per half so the wire overlaps the rest."""
                for q in range(4):
                    wt = wchunk(f"wo_{tag}_{q}", 4 * 1024)
                    nc.sync.dma_start(
                        wt[:], w_dram.ap()[:, 4096 * q:4096 * (q + 1)])
                    ps = [pp(f"ps_op_{tag}_{q}_{mm}", Lq) for mm in range(8)]
                    for k in range(4):
                        for mm in range(8):
                            nc.tensor.matmul(
                                ps[mm],
                                wt[:, 1024 * k + 128 * mm:
                                   1024 * k + 128 * (mm + 1)],
                                oT[k][:],
                                start=(k == 0), stop=(k == 3))
                    h = q // 2
                    for sub in range(2):
                        st = acts.tile([128, 4 * Lq], F16,
                                       name=f"st_{tag}_{q}_{sub}",
                                       tag="stage", bufs=3,
                                       padded_shape=[128, 4 * 512])
                        for mi in range(4):
                            mm = 4 * sub + mi
                            nc.vector.tensor_copy(
                                st[:, mi * Lq:(mi + 1) * Lq], ps[mm])
                        off = ((8 * (q % 2)) + 4 * sub) * Lq
                        nc.sync.dma_start(
                            ar_in_halves[h][:, off:off + 4 * Lq], st[:])
                    if q == 1 or q == 3:
                        nc.gpsimd.collective_compute(
                            "AllReduce", ALU.add, replica_groups=replica,
                            ins=[ar_in_halves[h].opt()],
                            outs=[ar_out_halves[h].opt()])

            def ln_finalize(tag, s1p, s2p, L):
                mean = acts.tile([1, L], F32, name=f"mean_{tag}", tag="lmean")
                var = acts.tile([1, L], F32, name=f"var_{tag}", tag="lvar")
                tmpa = acts.tile([1, L], F32, name=f"tmpa_{tag}", tag="ltmp")
                r0 = acts.tile([1, L], F32, name=f"r0_{tag}", tag="lr0")
                nc.scalar.mul(mean[:], s1p, 1.0 / E)
                nc.scalar.mul(var[:], s2p, 1.0 / E)
                nc.scalar.square(tmpa[:], mean[:])
                nc.vector.tensor_sub(var[:], var[:], tmpa[:])
                nc.vector.tensor_scalar_add(var[:], var[:], 1e-5)
                nc.scalar.sqrt(tmpa[:], var[:])
                nc.vector.reciprocal(r0[:], tmpa[:])
                nc.vector.tensor_tensor(tmpa[:], r0[:], r0[:], ALU.mult)
                nc.vector.tensor_tensor(tmpa[:], tmpa[:], var[:], ALU.mult)
                nc.vector.tensor_scalar(tmpa[:], tmpa[:], -0.5, 1.5, ALU.mult,
                                        ALU.add)
                rstd = acts.tile([1, L], F16, name=f"rstd_{tag}", tag="rstd")
                nmr = acts.tile([1, L], F16, name=f"nmr_{tag}", tag="nmr")
                nc.vector.tensor_tensor(rstd[:], r0[:], tmpa[:], ALU.mult)
                nc.vector.scalar_tensor_tensor(nmr[:], mean[:], -1.0, rstd[:],
                                               ALU.mult, ALU.mult)
                Apsum = pp(f"ps_A_{tag}", L)
                nc.tensor.matmul(Apsum, ones_row[:], rstd[:], start=True,
                                 stop=True)
                Bpsum = pp(f"ps_B_{tag}", L)
                nc.tensor.matmul(Bpsum, ones_row[:], nmr[:], start=True,
                                 stop=True)
                Asb = acts.tile([128, L], F16, name=f"A_{tag}", tag="Asb")
                nc.scalar.copy(Asb[:], Apsum)
                Bsb = acts.tile([128, L], F16, name=f"B_{tag}", tag="Bsb")
                nc.scalar.copy(Bsb[:], Bpsum)
                return rstd, nmr, Asb, Bsb

            def ln_apply(x_ts, L, Asb, Bsb):
                for k in range(ET):
                    nc.vector.tensor_tensor(xs(x_ts, L, k), xs(x_ts, L, k),
                                            Asb[:], ALU.mult)
                    nc.vector.tensor_tensor(xs(x_ts, L, k), xs(x_ts, L, k),
                                            Bsb[:], ALU.add)

            def residual_ln2(tag, ar_out_halves, x_ts, L, dump=None):
                s1p = pstat(f"ps_s1_{tag}", L)
                s2p = pstat(f"ps_s2_{tag}", L)
                CG = 4  # k-blocks per arb read chunk
                for g in range(ET // CG):
                    half, hoff = g // 4, (g % 4) * CG * L
                    b = acts.tile([128, CG * L], F16, name=f"arb_{tag}_{g}",
                                  tag="arb", bufs=3,
                                  padded_shape=[128, CG * 512])
                    nc.sync.dma_start(
                        b[:], ar_out_halves[half][:, hoff:hoff + CG * L])
                    gg, off = (CG * g) // KG, ((CG * g) % KG) * L
                    nc.vector.tensor_tensor(
                        x_ts[gg][:, off:off + CG * L], b[:],
                        x_ts[gg][:, off:off + CG * L], ALU.add)
                    for kk in range(CG):
                        k = CG * g + kk
                        nc.tensor.matmul(s1p, ones_col[:], xs(x_ts, L, k),
                                         start=(k == 0), stop=(k == ET - 1))
                    for kk in range(CG):
                        k = CG * g + kk
                        sq = acts.tile([128, L], F16, name=f"sq_{tag}_{k}",
                                       tag="sqt", bufs=3,
                                       padded_shape=[128, 512])
                        nc.scalar.square(sq[:], xs(x_ts, L, k))
                        nc.tensor.matmul(s2p, ones_col[:], sq[:],
                                         start=(k == 0), stop=(k == ET - 1))
                rstd, nmr, Asb, Bsb = ln_finalize(tag, s1p, s2p, L)
                ln_apply(x_ts, L, Asb, Bsb)
                if dump is not None:
                    for g in range(ET // KG):
                        nc.sync.dma_start(
                            dump.ap()[:, KG * L * g:KG * L * (g + 1)],
                            x_ts[g][:])

            # ================= program =================
            rem_ts = load_x("remx", remp_d, nrem, NG_R)
            cat_ts = load_x("catx", catp_d, ncat, NG_C)

            # ---- MHA2 (rem self-attention) ----
            q2 = proj_fm("q2", wd["q2"], rem_ts, nrem, "q")
            k2 = proj_fm("k2", wd["k2"], rem_ts, nrem, "k")
            v2 = proj_tm("v2", wd["v2"], rem_ts, nrem, JR)
            o2 = attention("a2", q2, k2, v2, nrem, nrem, JR)
            arin2 = [dram.tile([128, 16 * nrem], F16, name=f"arin2{h}",
                               tag=f"arin2{h}") for h in range(2)]
            arout2 = [dram.tile([128, 16 * nrem], F16, name=f"arout2{h}",
                                tag=f"arout2{h}", addr_space="Shared")
                      for h in range(2)]
            out_proj_to_dram("op2", o2, wd["o2"], arin2, arout2, nrem)

            # ---- MHA1 (cat self-attention), overlaps AR2 ----
            q1 = proj_fm("q1", wd["q1"], cat_ts, ncat, "q")
            k1 = proj_fm("k1", wd["k1"], cat_ts, ncat, "k")
            v1 = proj_tm("v1", wd["v1"], cat_ts, ncat, JC)
            o1 = attention("a1", q1, k1, v1, ncat, ncat, JC)
            arin1 = [dram.tile([128, 16 * ncat], F16, name=f"arin1{h}",
                               tag=f"arin1{h}") for h in range(2)]
            arout1 = [dram.tile([128, 16 * ncat], F16, name=f"arout1{h}",
                                tag=f"arout1{h}", addr_space="Shared")
                      for h in range(2)]
            out_proj_to_dram("op1", o1, wd["o1"], arin1, arout1, ncat)

            # ---- LN stages: r = LN(AR2 + rem); x1 = LN(AR1 + cat) ----
            # qc is emitted between the two LNs so the PE isn't blocked
            # behind x1's stats (which wait on AR1) while r is ready.
            residual_ln2("r", arout2, rem_ts, nrem, dump=dbg.get("dbg_r"))
            qc = proj_fm("qc", wd["qc"], rem_ts, nrem, "q")
            residual_ln2("x1", arout1, cat_ts, ncat, dump=dbg.get("dbg_x1"))

            # ---- MHAc (q from r, kv from x1) ----
            kc = proj_fm("kc", wd["kc"], cat_ts, ncat, "k")
            vc = proj_tm("vc", wd["vc"], cat_ts, ncat, JC)
            oc = attention("ac", qc, kc, vc, nrem, ncat, JC)
            arinc = [dram.tile([128, 16 * nrem], F16, name=f"arinc{h}",
                               tag=f"arinc{h}") for h in range(2)]
            aroutc = [dram.tile([128, 16 * nrem], F16, name=f"aroutc{h}",
                                tag=f"aroutc{h}", addr_space="Shared")
                      for h in range(2)]
            out_proj_to_dram("opc", oc, wd["oc"], arinc, aroutc, nrem)

            # ---- LN x2 commuted into f1 ----
            # f1 projects the raw residual xsum2 = r + ARc and chases the
            # two ARc halves (no wait for the full AllReduce or the LN);
            # the per-token LN scale/shift commutes through the linear
            # projection and is applied to the f1 pre-activations via
            # A * f1_u + (Wf1 . 1) (x) B before the gelu.
            s1p2 = pstat("ps_s1_x2", nrem)
            s2p2 = pstat("ps_s2_x2", nrem)

            def x2_addstats_half(h):
                CG = 4
                for g in range(4 * h, 4 * (h + 1)):
                    b = acts.tile([128, CG * nrem], F16, name=f"arb_x2_{g}",
                                  tag="arb", bufs=3,
                                  padded_shape=[128, CG * 512])
                    hoff = (g % 4) * CG * nrem
                    nc.sync.dma_start(
                        b[:], aroutc[h][:, hoff:hoff + CG * nrem])
                    gg, off = (CG * g) // KG, ((CG * g) % KG) * nrem
                    nc.vector.tensor_tensor(
                        rem_ts[gg][:, off:off + CG * nrem], b[:],
                        rem_ts[gg][:, off:off + CG * nrem], ALU.add)
                    for kk in range(CG):
                        k = CG * g + kk
                        nc.tensor.matmul(s1p2, ones_col[:],
                                         xs(rem_ts, nrem, k),
                                         start=(k == 0), stop=(k == ET - 1))
                    for kk in range(CG):
                        k = CG * g + kk
                        sq = acts.tile([128, nrem], F16, name=f"sq_x2_{k}",
                                       tag="sqt", bufs=3,
                                       padded_shape=[128, 512])
                        nc.scalar.square(sq[:], xs(rem_ts, nrem, k))
                        nc.tensor.matmul(s2p2, ones_col[:], sq[:],
                                         start=(k == 0), stop=(k == ET - 1))

            def f1_pass(mlo, ps_list, klo, khi, tagx):
                for g in range(klo // KG, (khi + KG - 1) // KG):
                    wt = wchunk(f"w_f1_{tagx}_{g}", KG * FLOC)
                    nc.sync.dma_start(
                        wt[:],
                        wd["f1"].ap()[:, KG * FLOC * g:KG * FLOC * (g + 1)])
                    for kk in range(KG):
                        k = KG * g + kk
                        if not (klo <= k < khi):
                            continue
                        for mi in range(4):
                            m = mlo + mi
                            nc.tensor.matmul(
                                ps_list[mi],
                                wt[:, kk * FLOC + 128 * m:
                                   kk * FLOC + 128 * (m + 1)],
                                xs(rem_ts, nrem, k),
                                start=(k == 0), stop=(k == ET - 1))

            def f1_correct(mlo, ps_list, Asb, nmr):
                out = []
                for mi in range(4):
                    m = mlo + mi
                    u = acts.tile([128, nrem], F16, name=f"f1u_{m}",
                                  tag="f1u", bufs=2, padded_shape=[128, 512])
                    nc.vector.tensor_tensor(u[:], ps_list[mi], Asb[:],
                                            ALU.mult)
                    opp = pp(f"ps_f1o_{m}", nrem)
                    nc.tensor.matmul(opp, f1rs_sb[:, 128 * m:128 * (m + 1)],
                                     nmr[:], start=True, stop=True)
                    nc.vector.tensor_tensor(u[:], u[:], opp, ALU.add)
                    h = acts.tile([128, nrem], F16, name=f"hT_{m}",
                                  tag=f"hT_{m}")
                    nc.scalar.activation(h[:], u[:], AF.Gelu)
                    out.append(h)
                return out

            x2_addstats_half(0)
            psA = [pp(f"ps_f1a_{m}", nrem) for m in range(4)]
            f1_pass(0, psA, 0, 16, "a0")
            x2_addstats_half(1)
            wxu = pstat("ps_wx2", nrem)
            for k in range(ET):
                nc.tensor.matmul(wxu, ws_sb[:, k:k + 1],
                                 xs(rem_ts, nrem, k),
                                 start=(k == 0), stop=(k == ET - 1))
            f1_pass(0, psA, 16, 32, "a1")
            rstd2, nmr2, A2sb, B2sb = ln_finalize("x2", s1p2, s2p2, nrem)
            wx2 = acts.tile([1, nrem], F32, name="wx2", tag="wx2")
            nc.vector.tensor_tensor(wx2[:], wxu, rstd2[:], ALU.mult)
            nc.vector.scalar_tensor_tensor(wx2[:], nmr2[:],
                                           consts[0:1, 0:1], wx2[:],
                                           ALU.mult, ALU.add)
            hT = f1_correct(0, psA, A2sb, nmr2)
            psB = [pp(f"ps_f1b_{m}", nrem) for m in range(4)]
            f1_pass(4, psB, 0, 16, "b0")
            # materialize x2 in place (f2 residual fold needs it); each
            # half's apply is emitted after group B's reads of that half
            # so the in-place overwrite of xsum2 is safe
            for k in range(16):
                nc.vector.tensor_tensor(xs(rem_ts, nrem, k),
                                        xs(rem_ts, nrem, k), A2sb[:],
                                        ALU.mult)
                nc.vector.tensor_tensor(xs(rem_ts, nrem, k),
                                        xs(rem_ts, nrem, k), B2sb[:],
                                        ALU.add)
            f1_pass(4, psB, 16, 32, "b1")
            for k in range(16, ET):
                nc.vector.tensor_tensor(xs(rem_ts, nrem, k),
                                        xs(rem_ts, nrem, k), A2sb[:],
                                        ALU.mult)
                nc.vector.tensor_tensor(xs(rem_ts, nrem, k),
                                        xs(rem_ts, nrem, k), B2sb[:],
                                        ALU.add)
            hT += f1_correct(4, psB, A2sb, nmr2)

            # ---- linear logit stats from hT: s1 = c2.g ; ws += w2s.g/256 ----
            c2p = pstat("ps_c2", nrem)
            w2p = pstat("ps_w2s", nrem)
            for m in range(8):
                nc.tensor.matmul(c2p, c2w_sb[:, m:m + 1], hT[m][:],
                                 start=(m == 0), stop=(m == 7))
            for m in range(8):
                nc.tensor.matmul(w2p, c2w_sb[:, 8 + m:9 + m], hT[m][:],
                                 start=(m == 0), stop=(m == 7))
            # stat staging rows padded to 512 so each maps onto a [128, 4]
            # block of the tiny-AR tensor (tail math then runs 128-wide)
            s1part = acts.tile([1, 512], F32, name="s1part", tag="s1part")
            wspart = acts.tile([1, 512], F32, name="wspart", tag="wspart")
            nc.vector.memset(s1part[:], 1.0)
            nc.vector.memset(wspart[:], 0.0)
            nc.vector.tensor_copy(s1part[:, 0:nrem], c2p)
            nc.vector.tensor_scalar(wx2[:], wx2[:], 1.0 / NCORES, 0.0,
                                    ALU.mult, ALU.add)
            nc.vector.scalar_tensor_tensor(wspart[:, 0:nrem], w2p,
                                           1.0 / 256.0, wx2[:],
                                           ALU.mult, ALU.add)
            arin6 = dram.tile([128, 16], F32, name="arin6", tag="arin6")
            arout6 = dram.tile([128, 16], F32, name="arout6", tag="arout6",
                               addr_space="Shared")
            nc.sync.dma_start(arin6[:, 0:4], s1part[:])
            nc.sync.dma_start(arin6[:, 4:8], wspart[:])
            nc.sync.dma_start(arin6[:, 12:16], s1part[:])

            # ---- FFN f2: partial = x2/8 + Wf2_shard^T hT; RS per half ----
            arin4 = [dram.tile([128, 16 * nrem], F16, name=f"arin4{h}",
                               tag=f"arin4{h}") for h in range(2)]
            rs4 = [dram.tile([16, 16 * nrem], F16, name=f"rs4{h}",
                             tag=f"rs4{h}") for h in range(2)]
            for q in range(4):
                wt = wchunk(f"w_f2_{q}", 8 * 1024)
                nc.sync.dma_start(
                    wt[:], wd["f2"].ap()[:, 8192 * q:8192 * (q + 1)])
                ps = [pp(f"ps_f2_{q}_{mm}", nrem) for mm in range(8)]
                for k in range(8):
                    for mm in range(8):
                        nc.tensor.matmul(
                            ps[mm],
                            wt[:, 1024 * k + 128 * mm:
                               1024 * k + 128 * (mm + 1)],
                            hT[k][:],
                            start=(k == 0), stop=(k == 7))
                for sub in range(2):
                    st = acts.tile([128, 4 * nrem], F16,
                                   name=f"st_f2_{q}_{sub}",
                                   tag="stage", bufs=3,
                                   padded_shape=[128, 4 * 512])
                    for mi in range(4):
                        mm = 4 * sub + mi
                        m = 8 * q + mm
                        nc.vector.scalar_tensor_tensor(
                            st[:, mi * nrem:(mi + 1) * nrem],
                            xs(rem_ts, nrem, m), 1.0 / NCORES, ps[mm],
                            ALU.mult, ALU.add)
                    off = ((8 * (q % 2)) + 4 * sub) * nrem
                    nc.sync.dma_start(
                        arin4[q // 2][:, off:off + 4 * nrem], st[:])
                if q == 1 or q == 3:
                    nc.gpsimd.collective_compute(
                        "ReduceScatter", ALU.add, replica_groups=replica,
                        ins=[arin4[q // 2].opt()],
                        outs=[rs4[q // 2].opt()])

            # ---- s2 from the scattered summed features ----
            NCH = 8
            CW = ET * nrem // NCH  # columns per rs4 read chunk
            NSUB = CW // nrem  # k-subblocks per chunk
            s2p = pstat("ps_rs2", nrem)
            for cch in range(NCH):
                hh, hcol = cch // 4, (cch % 4) * CW
                bt = acts.tile([16, CW], F16, name=f"rsb_{cch}", tag="rsb",
                               bufs=2, padded_shape=[16, 4 * 512])
                nc.sync.dma_start(bt[:], rs4[hh][:, hcol:hcol + CW])
                sq = acts.tile([16, CW], F16, name=f"rssq_{cch}", tag="rssq",
                               bufs=2, padded_shape=[16, 4 * 512])
                nc.scalar.square(sq[:], bt[:])
                for s in range(NSUB):
                    k = cch * NSUB + s
                    nc.tensor.matmul(s2p, ones_col[0:16, :],
                                     sq[:, s * nrem:(s + 1) * nrem],
                                     start=(k == 0), stop=(k == ET - 1))
            s2part = acts.tile([1, 512], F32, name="s2part", tag="s2part")
            nc.vector.memset(s2part[:], 1.0)
            nc.vector.tensor_copy(s2part[:, 0:nrem], s2p)
            nc.sync.dma_start(arin6[:, 8:12], s2part[:])
            nc.gpsimd.collective_compute(
                "AllReduce", ALU.add, replica_groups=replica,
                ins=[arin6.opt()], outs=[arout6.opt()])

            # ---- final logit: affine-LN identity, 128-wide blocked ----
            gsb = acts.tile([128, 12], F32, name="gsb", tag="gsb")
            nc.sync.dma_start(gsb[:], arout6[:, 0:12])
            g1, g2, g3 = gsb[:, 0:4], gsb[:, 4:8], gsb[:, 8:12]
            mean = acts.tile([128, 4], F32, name="mean_l", tag="lmean")
            var = acts.tile([128, 4], F32, name="var_l", tag="lvar")
            tmpa = acts.tile([128, 4], F32, name="tmpa_l", tag="ltmp")
            r0 = acts.tile([128, 4], F32, name="r0_l", tag="lr0")
            nc.scalar.mul(mean[:], g1, 1.0 / E)
            nc.scalar.mul(var[:], g3, 1.0 / E)
            nc.scalar.square(tmpa[:], mean[:])
            nc.vector.tensor_sub(var[:], var[:], tmpa[:])
            nc.vector.tensor_scalar_add(var[:], var[:], 1e-5)
            nc.scalar.sqrt(tmpa[:], var[:])
            nc.vector.reciprocal(r0[:], tmpa[:])
            nc.vector.tensor_tensor(tmpa[:], r0[:], r0[:], ALU.mult)
            nc.vector.tensor_tensor(tmpa[:], tmpa[:], var[:], ALU.mult)
            nc.vector.tensor_scalar(tmpa[:], tmpa[:], -0.5, 1.5,
                                    ALU.mult, ALU.add)
            rstd = acts.tile([128, 4], F32, name="rstd_l", tag="rstd_l")
            nc.vector.tensor_tensor(rstd[:], r0[:], tmpa[:], ALU.mult)
            nmr = acts.tile([128, 4], F32, name="nmr_l", tag="nmr_l")
            nc.vector.scalar_tensor_tensor(nmr[:], mean[:], -1.0,
                                           rstd[:], ALU.mult, ALU.mult)
            wdot = acts.tile([128, 4], F32, name="wdot", tag="wdot")
            nc.vector.tensor_tensor(wdot[:], rstd[:], g2, ALU.mult)
            lsb = acts.tile([128, 4], F32, name="lsb", tag="lsb")
            nc.vector.scalar_tensor_tensor(lsb[:], nmr[:],
                                           consts[:, 0:1], wdot[:],
                                           ALU.mult, ALU.add)
            nc.sync.dma_start(logits_d.ap(), lsb[:])

    nc.compile()
    return nc


# ----------------------------------------------------------------------------
# host orchestration
# ----------------------------------------------------------------------------

def _packx(XT):
    """[E, L] fp32 -> [128, ET*L] fp16 feature-block pack."""
    L = XT.shape[1]
    return np.ascontiguousarray(
        XT.reshape(ET, 128, L).transpose(1, 0, 2).reshape(128, ET * L)
        .astype(np.float16))


def _prep_in_maps(vision_feature, text_embed, sel_idx, rem_idx,
                  Wqkv1, Wo1, Wqkv2, Wo2, Wqkvc, Woc, Wf1, Wf2, Ws):
    f16 = np.float16
    sel = vision_feature[sel_idx]
    rem = vision_feature[rem_idx]
    cat = np.concatenate([sel, text_embed], axis=0)

    remp = _packx(np.ascontiguousarray(rem.T))
    catp = _packx(np.ascontiguousarray(cat.T))
    ws_pack = np.ascontiguousarray(Ws[0].reshape(ET, 128).T.astype(f16))
    consts = np.broadcast_to(
        np.array([[np.float64(Ws.astype(np.float64).sum()), 0.0]],
                 np.float32), (128, 2)).copy()

    in_maps = []
    for c in range(NCORES):
        hs = slice(DLOC * c, DLOC * (c + 1))
        fs = slice(FLOC * c, FLOC * (c + 1))
        m = {"remp": remp, "catp": catp, "wsp": ws_pack, "consts": consts}
        for l, Wqkv, Wo in (("1", Wqkv1, Wo1), ("2", Wqkv2, Wo2),
                            ("c", Wqkvc, Woc)):
            Wq, Wk, Wv = Wqkv[:E], Wqkv[E:2 * E], Wqkv[2 * E:]
            for nm, W in (("q", Wq), ("k", Wk), ("v", Wv)):
                A = W[hs].T  # [E, DLOC]
                m[f"w{nm}{l}"] = np.ascontiguousarray(
                    A.reshape(ET, 128, DLOC).transpose(1, 0, 2)
                    .reshape(128, ET * DLOC).astype(f16))
            WoT = Wo[:, hs].T  # [DLOC, E]
            m[f"wo{l}"] = np.ascontiguousarray(
                WoT.reshape(4, 128, 4, 1024).transpose(1, 2, 0, 3)
                .reshape(128, 4 * E).astype(f16))
        A = Wf1[fs].T  # [E, FLOC]
        m["wf1"] = np.ascontiguousarray(
            A.reshape(ET, 128, FLOC).transpose(1, 0, 2)
            .reshape(128, ET * FLOC).astype(f16))
        W2T = Wf2[:, fs].T  # [FLOC, E]
        m["wf2"] = np.ascontiguousarray(
            W2T.reshape(8, 128, 4, 1024).transpose(1, 2, 0, 3)
            .reshape(128, 8 * E).astype(f16))
        c2 = Wf2[:, fs].astype(np.float64).sum(axis=0)  # [FLOC]
        w2s = 256.0 * (Ws[0].astype(np.float64) @ Wf2[:, fs].astype(np.float64))
        c2w = np.concatenate([c2.reshape(8, 128).T, w2s.reshape(8, 128).T],
                             axis=1)  # [128, 16]
        m["c2w"] = np.ascontiguousarray(c2w.astype(f16))
        m["f1rs"] = np.ascontiguousarray(
            Wf1[fs].astype(np.float64).sum(axis=1).reshape(1, FLOC)
            .astype(f16))
        in_maps.append(m)
    return in_maps


def run_device(in_maps, ncat_real, nrem_real, dumps=False, trace=False):
    from concourse.bass_utils import run_bass_kernel_spmd

    key = (ncat_real, nrem_real, dumps)
    if key not in _CACHE:
        _CACHE[key] = _build_device(ncat_real, nrem_real, dumps=dumps)
    nc = _CACHE[key]
    return run_bass_kernel_spmd(nc, in_maps, list(range(NCORES)), trace=trace)


def _kernel_impl(inputs, debug=False, trace=False):
    vision_feature = np.asarray(inputs["vision_feature"], np.float32)
    text_embed = np.asarray(inputs["text_embed"], np.float32)
    attention_mask = np.asarray(inputs["attention_mask"])

    biases_zero = all(
        not np.any(np.asarray(inputs[b]))
        for b in ("bqkv1", "bo1", "bqkv2", "bo2", "bqkvc", "boc",
                  "bf1", "bf2", "bs"))
    if (not bool(attention_mask.all())) or (not biases_zero):
        return (_reference_np(**{k: np.asarray(v) for k, v in inputs.items()}),
                None)

    t, sel_idx, rem_idx = _score_partition(vision_feature, text_embed,
                                           attention_mask)
    ncat_real = t + text_embed.shape[0]
    nrem_real = vision_feature.shape[0] - t
    kk = int(t * EXPAND)

    in_maps = _prep_in_maps(
        vision_feature, text_embed, sel_idx, rem_idx,
        np.asarray(inputs["Wqkv1"], np.float32),
        np.asarray(inputs["Wo1"], np.float32),
        np.asarray(inputs["Wqkv2"], np.float32),
        np.asarray(inputs["Wo2"], np.float32),
        np.asarray(inputs["Wqkvc"], np.float32),
        np.asarray(inputs["Woc"], np.float32),
        np.asarray(inputs["Wf1"], np.float32),
        np.asarray(inputs["Wf2"], np.float32),
        np.asarray(inputs["Ws"], np.float32))
    res = run_device(in_maps, ncat_real, nrem_real, dumps=debug, trace=trace)
    logits = res.results[0]["logits"][0, :nrem_real]
    es = (1.0 / (1.0 + np.exp(-logits.astype(np.float32))))
    ei = np.argsort(-es, kind="stable")[:kk]
    final = np.sort(np.concatenate([sel_idx, rem_idx[ei]]))
    return vision_feature[final], res


def kernel(**inputs):
    out, _ = _kernel_impl(inputs)
    return out



# revision 2
# speedup vs baseline: 1.0056x; 1.0056x over previous
"""Trainium2 Bass kernel for nn_CosSimRouter_learn_49778670960796. v2.

Schedule-restructured vs v1:
  * All big collectives quartered (one AR per out-proj feature quarter)
    and fired as soon as each quarter's staging lands; every consumer
    chases quarter-wise.
  * Program order: MHA1(cat) first (AR1 rides the empty early CC
    window), then MHA2(rem), then kc/vc, then qc which projects the RAW
    rem residual (r pre-LN) chasing AR2 quarters; the per-token LN
    scale/shift is commuted through the linear projection and applied
    as qc = rstd (x) qc_u + (Wq^T 1) (x) nmr afterwards (rank-1 via
    matmul + DVE).
  * LN statistics are accumulated on DVE/Pool/Scalar tile-wise while
    quarters land (no 32x ones-matmul chains on the PE): sacc (Pool,
    fp16), sq (Scalar) + qacc (DVE, fp32), then a single ones-matmul
    partition-reduce each.  Ws.x2sum (wx2) likewise via Scalar
    per-partition-scale copy + Pool fp32 accumulate.
  * f1 chases ARc quarters 6 psum banks wide (m0-5), m6/m7 run
    back-to-back after the last quarter; PSUM ring split 6 ("pp") + 2
    ("pps" for [1,L] stats / LN broadcast psums).
  * f2 fires one ReduceScatter per output-feature quarter; the s2
    stat chase + tiny fp32 AR (s1/ws/s2) close the kernel.
"""

import numpy as np

E = 4096
H = 16
HID = 8192
GAMMA = 0.2
TEMP = 0.05
EXPAND = 0.7
NCORES = 8
ET = E // 128  # 32 feature tiles
DH = E // H  # 256
HL = H // NCORES  # 2 heads per core
DLOC = HL * DH  # 512 local head dims
FLOC = HID // NCORES  # 1024 local ffn hidden
KG = 8  # k-blocks per weight/act chunk (== one AR quarter)

_CACHE = {}


# ----------------------------------------------------------------------------
# host-side reference math (numpy, fp32) for the scoring stage + fallback
# ----------------------------------------------------------------------------

def _score_partition(vision_feature, text_embed, attention_mask):
    vf = vision_feature.astype(np.float32)
    te = text_embed.astype(np.float32)
    vn = vf / np.maximum(np.linalg.norm(vf, axis=-1, keepdims=True), 1e-8)
    tn = te / np.maximum(np.linalg.norm(te, axis=-1, keepdims=True), 1e-8)
    cs = vn @ tn.T
    cs = np.where(attention_mask[None, :], cs, np.float32(0.0))
    m = cs.max(axis=-1) / np.float32(TEMP)
    e = np.exp(m - m.max())
    scores = e / e.sum()
    order = np.argsort(-scores, kind="stable")
    cum = np.cumsum(scores[order])
    t = int((cum <= GAMMA).sum())
    return t, order[:t], order[t:]


def _ln_np(x):
    m = x.mean(-1, keepdims=True)
    v = ((x - m) ** 2).mean(-1, keepdims=True)
    return (x - m) / np.sqrt(v + 1e-5)


def _gelu_np(x):
    import math

    erf = np.frompyfunc(math.erf, 1, 1)
    return (x * 0.5 * (1.0 + erf(x / math.sqrt(2.0)).astype(np.float64))
            ).astype(x.dtype)


def _mha_np(q_in, kv_in, Wqkv, bqkv, Wo, bo):
    dh = E // H
    Wq, Wk, Wv = np.split(Wqkv, 3, axis=0)
    bq, bk, bv = np.split(bqkv, 3)
    q = (q_in @ Wq.T + bq).reshape(-1, H, dh)
    k = (kv_in @ Wk.T + bk).reshape(-1, H, dh)
    v = (kv_in @ Wv.T + bv).reshape(-1, H, dh)
    att = np.einsum("qhd,khd->hqk", q, k) / np.float32(np.sqrt(dh))
    att = att - att.max(-1, keepdims=True)
    att = np.exp(att)
    att /= att.sum(-1, keepdims=True)
    o = np.einsum("hqk,khd->qhd", att.astype(np.float32), v).reshape(-1, E)
    return o @ Wo.T + bo


def _reference_np(vision_feature, text_embed, attention_mask,
                  Wqkv1, bqkv1, Wo1, bo1, Wqkv2, bqkv2, Wo2, bo2,
                  Wqkvc, bqkvc, Woc, boc, Wf1, bf1, Wf2, bf2, Ws, bs):
    t, sel_idx, rem_idx = _score_partition(vision_feature, text_embed,
                                           attention_mask)
    sel = vision_feature[sel_idx]
    rem = vision_feature[rem_idx]
    cat = np.concatenate([sel, text_embed], axis=0)
    x = _ln_np(_mha_np(cat, cat, Wqkv1, bqkv1, Wo1, bo1) + cat)
    r = _ln_np(_mha_np(rem, rem, Wqkv2, bqkv2, Wo2, bo2) + rem)
    x = _ln_np(_mha_np(r, x, Wqkvc, bqkvc, Woc, boc) + r)
    ffn = _gelu_np(x @ Wf1.T + bf1) @ Wf2.T + bf2
    x = _ln_np(x + ffn)
    logits = (x @ Ws.T + bs).squeeze(-1)
    es = 1.0 / (1.0 + np.exp(-logits))
    k = int(t * EXPAND)
    ei = np.argsort(-es, kind="stable")[:k]
    final = np.sort(np.concatenate([sel_idx, rem_idx[ei]]))
    return vision_feature[final]


# ----------------------------------------------------------------------------
# device program
# ----------------------------------------------------------------------------

def _build_device(ncat, nrem, dumps=False):
    import concourse.bacc as bacc
    import concourse.mybir as mybir
    import concourse.tile as tile

    dt = mybir.dt
    F32 = dt.float32
    F16 = dt.float16
    AF = mybir.ActivationFunctionType
    ALU = mybir.AluOpType

    JC = (ncat + 127) // 128  # kv partition tiles for cat (2)
    JR = (nrem + 127) // 128  # kv partition tiles for rem (4)

    nc = bacc.Bacc("TRN2", target_bir_lowering=False, debug=False,
                   num_devices=NCORES)

    # ---------------- DRAM I/O (all host-packed, see _prep_in_maps) --------
    remp_d = nc.dram_tensor("remp", [128, ET * nrem], F16, kind="ExternalInput")
    catp_d = nc.dram_tensor("catp", [128, ET * ncat], F16, kind="ExternalInput")
    wd = {}
    for l in ("1", "2", "c"):
        for p in ("q", "k", "v"):
            wd[p + l] = nc.dram_tensor(f"w{p}{l}", [128, ET * DLOC], F16,
                                       kind="ExternalInput")
        wd["o" + l] = nc.dram_tensor(f"wo{l}", [128, (DLOC // 128) * E], F16,
                                     kind="ExternalInput")
    wd["f1a"] = nc.dram_tensor("wf1a", [128, ET * 768], F16,
                               kind="ExternalInput")
    wd["f1b"] = nc.dram_tensor("wf1b", [128, ET * 256], F16,
                               kind="ExternalInput")
    wd["f2"] = nc.dram_tensor("wf2", [128, (FLOC // 128) * E], F16,
                              kind="ExternalInput")
    ws_d = nc.dram_tensor("wsp", [128, ET], F16, kind="ExternalInput")
    c2w_d = nc.dram_tensor("c2w", [128, 2 * (FLOC // 128)], F16,
                           kind="ExternalInput")
    f1rs_d = nc.dram_tensor("f1rs", [1, FLOC], F16, kind="ExternalInput")
    wq1r_d = nc.dram_tensor("wq1r", [1, DLOC], F16, kind="ExternalInput")
    consts_d = nc.dram_tensor("consts", [128, 2], F32, kind="ExternalInput")
    logits_d = nc.dram_tensor("logits", [1, 512], F32, kind="ExternalOutput")
    dbg = {}
    if dumps:
        for nm, L in (("dbg_x1", ncat), ("dbg_r", nrem), ("dbg_x2", nrem)):
            dbg[nm] = nc.dram_tensor(nm, [128, ET * L], F16,
                                     kind="ExternalOutput")

    replica = [list(range(NCORES))]

    with tile.TileContext(nc, num_cores=NCORES) as tc:
        with (
            tc.tile_pool(name="acts", bufs=1) as acts,
            tc.tile_pool(name="psum", bufs=1, space="PSUM") as psum,
            tc.tile_pool(name="dram", bufs=1, space="DRAM") as dram,
        ):
            # ---- constants ----
            ones_col = acts.tile([128, 1], F16, name="ones_col",
                                 tag="ones_col")
            nc.vector.memset(ones_col[:], 1.0)
            ones_row = acts.tile([1, 128], F16, name="ones_row",
                                 tag="ones_row")
            nc.vector.memset(ones_row[:], 1.0)
            ws_sb = acts.tile([128, ET], F16, name="ws_sb", tag="ws_sb")
            nc.sync.dma_start(ws_sb[:], ws_d.ap())
            ws32 = acts.tile([128, ET], F32, name="ws32", tag="ws32")
            nc.vector.tensor_copy(ws32[:], ws_sb[:])
            c2w_sb = acts.tile([128, 2 * (FLOC // 128)], F16, name="c2w_sb",
                               tag="c2w_sb")
            nc.sync.dma_start(c2w_sb[:], c2w_d.ap())
            consts = acts.tile([128, 2], F32, name="consts", tag="consts")
            nc.sync.dma_start(consts[:], consts_d.ap())
            f1rs_sb = acts.tile([1, FLOC], F16, name="f1rs_sb", tag="f1rs")
            nc.sync.dma_start(f1rs_sb[:], f1rs_d.ap())
            wq1r_sb = acts.tile([1, DLOC], F16, name="wq1r_sb", tag="wq1r")
            nc.sync.dma_start(wq1r_sb[:], wq1r_d.ap())

            # PSUM ring: 6 full banks ("pp") + 2 banks for [1,L] stats and
            # LN broadcast psums ("pps"). 6*2048 + 2*2048 = 16 KiB.
            def pp(name, L, parts=128):
                t_ = psum.tile([128, L], F32, name=name, tag="pp", bufs=6,
                               padded_shape=[128, 512])
                return t_[0:parts, :] if parts < 128 else t_[:]

            def pstat(name, L):
                return psum.tile([1, L], F32, name=name, tag="pps", bufs=1,
                                 padded_shape=[1, 512])[:]

            def pb(name, L):
                return psum.tile([128, L], F32, name=name, tag="ppb",
                                 bufs=1, padded_shape=[128, 512])[:]

            def wchunk(name, cols):
                return acts.tile([128, cols], F16, name=name, tag="wt",
                                 bufs=3, padded_shape=[128, 4096])

            # ---- activations: group tiles + slice helper ----
            def load_x(name, dram_t, L, ngroups):
                ts = []
                for g in range(ngroups):
                    xt = acts.tile([128, KG * L], F16, name=f"{name}_{g}",
                                   tag=f"{name}_{g}")
                    nc.sync.dma_start(
                        xt[:], dram_t.ap()[:, KG * L * g:KG * L * (g + 1)])
                    ts.append(xt)
                return ts

            def xs(ts, L, k):
                g, kk = k // KG, k % KG
                return ts[g][:, kk * L:(kk + 1) * L]

            # ---------------- building blocks ----------------
            def proj_fm(tagbase, w_dram, x_ts, L, outtag, correct=None):
                """q/k projection -> 4 tiles [128, L] fp16 (DLOC, L) layout.

                correct=(Asb, nmr_row, w1r_sb): instead of a plain PSUM
                copy, apply the commuted-LN fixup
                out_m = Asb (x) psum_m + (w1r_m (x) nmr).
                """
                chunks = []
                for g in range(ET // KG):
                    wt = wchunk(f"w_{tagbase}_{g}", KG * DLOC)
                    nc.sync.dma_start(
                        wt[:],
                        w_dram.ap()[:, KG * DLOC * g:KG * DLOC * (g + 1)])
                    chunks.append(wt)
                ps = [pp(f"ps_{tagbase}_{m}", L) for m in range(4)]
                for k in range(ET):
                    g, kk = k // KG, k % KG
                    for m in range(4):
                        nc.tensor.matmul(
                            ps[m],
                            chunks[g][:, kk * DLOC + 128 * m:
                                      kk * DLOC + 128 * (m + 1)],
                            xs(x_ts, L, k),
                            start=(k == 0), stop=(k == ET - 1))
                outs = []
                if correct is None:
                    for m in range(4):
                        o = acts.tile([128, L], F16, name=f"{tagbase}_{m}",
                                      tag=f"{outtag}_{m}")
                        nc.scalar.copy(o[:], ps[m])
                        outs.append(o)
                else:
                    Asb, nmr_row, w1r = correct
                    for m in range(4):
                        r1p = pp(f"ps_r1_{tagbase}_{m}", L)
                        nc.tensor.matmul(r1p,
                                         w1r[:, 128 * m:128 * (m + 1)],
                                         nmr_row[:], start=True, stop=True)
                        o = acts.tile([128, L], F16, name=f"{tagbase}_{m}",
                                      tag=f"{outtag}_{m}")
                        nc.vector.tensor_tensor(o[:], ps[m], Asb[:], ALU.mult)
                        nc.vector.tensor_tensor(o[:], o[:], r1p, ALU.add)
                        outs.append(o)
                return outs

            def proj_tm(tagbase, w_dram, x_ts, L, JT):
                """v projection -> JT tiles [128, DLOC] fp16 (kv, DLOC)."""
                chunks = []
                for g in range(ET // KG):
                    wt = wchunk(f"w_{tagbase}_{g}", KG * DLOC)
                    nc.sync.dma_start(
                        wt[:],
                        w_dram.ap()[:, KG * DLOC * g:KG * DLOC * (g + 1)])
                    chunks.append(wt)
                ps = []
                for j in range(JT):
                    pj = min(128, L - 128 * j)
                    ps.append(pp(f"ps_{tagbase}_{j}", DLOC, parts=pj))
                for k in range(ET):
                    g, kk = k // KG, k % KG
                    for j in range(JT):
                        pj = min(128, L - 128 * j)
                        nc.tensor.matmul(
                            ps[j],
                            xs(x_ts, L, k)[:, 128 * j:128 * j + pj],
                            chunks[g][:, kk * DLOC:(kk + 1) * DLOC],
                            start=(k == 0), stop=(k == ET - 1))
                outs = []
                for j in range(JT):
                    pj = min(128, L - 128 * j)
                    o = acts.tile([128, DLOC], F16, name=f"{tagbase}_{j}",
                                  tag=f"v_{j}")
                    nc.scalar.copy(o[0:pj, :], ps[j])
                    outs.append(o)
                return outs

            def attention(tag, qT, kT, vT, Lq, Lkv, JT):
                exps_h = []
                for h in range(HL):
                    exps = []
                    for j in range(JT):
                        pj = min(128, Lkv - 128 * j)
                        p = pp(f"ps_s_{tag}_{h}_{j}", Lq, parts=pj)
                        for c in range(2):
                            nc.tensor.matmul(
                                p,
                                kT[2 * h + c][:, 128 * j:128 * j + pj],
                                qT[2 * h + c][:],
                                start=(c == 0), stop=(c == 1))
                        e = acts.tile([128, Lq], F16,
                                      name=f"es_{tag}_{h}_{j}",
                                      tag=f"expS_{h}_{j}")
                        nc.scalar.activation(e[0:pj, :], p, AF.Exp,
                                             scale=float(1.0 / np.sqrt(DH)))
                        exps.append(e)
                    exps_h.append(exps)
                rec2s = []
                for h in range(HL):
                    dsum = pstat(f"ps_d_{tag}_{h}", Lq)
                    for j in range(JT):
                        pj = min(128, Lkv - 128 * j)
                        nc.tensor.matmul(dsum, ones_col[0:pj, :],
                                         exps_h[h][j][0:pj, :],
                                         start=(j == 0), stop=(j == JT - 1))
                    den = acts.tile([1, Lq], F32, name=f"den_{tag}_{h}",
                                    tag="aden")
                    rec = acts.tile([1, Lq], F32, name=f"rec_{tag}_{h}",
                                    tag="arec")
                    nc.vector.tensor_copy(den[:], dsum)
                    nc.vector.reciprocal(rec[:], den[:])
                    nc.vector.tensor_tensor(den[:], den[:], rec[:], ALU.mult)
                    nc.vector.tensor_scalar(den[:], den[:], -1.0, 2.0,
                                            ALU.mult, ALU.add)
                    rec2 = acts.tile([1, Lq], F16, name=f"rec2_{tag}_{h}",
                                     tag=f"rec2_{h}")
                    nc.vector.tensor_tensor(rec2[:], rec[:], den[:], ALU.mult)
                    rec2s.append(rec2)
                pos = []
                for h in range(HL):
                    for c in range(2):
                        po = pp(f"ps_o_{tag}_{h}_{c}", Lq)
                        for j in range(JT):
                            pj = min(128, Lkv - 128 * j)
                            nc.tensor.matmul(
                                po,
                                vT[j][0:pj, 256 * h + 128 * c:
                                      256 * h + 128 * (c + 1)],
                                exps_h[h][j][0:pj, :],
                                start=(j == 0), stop=(j == JT - 1))
                        pos.append(po)
                oT = []
                for h in range(HL):
                    rrep_p = pp(f"ps_rr_{tag}_{h}", Lq)
                    nc.tensor.matmul(rrep_p, ones_row[:], rec2s[h][:],
                                     start=True, stop=True)
                    rrep = acts.tile([128, Lq], F32, name=f"rr_{tag}_{h}",
                                     tag=f"rrep_{h}")
                    nc.scalar.copy(rrep[:], rrep_p)
                    for c in range(2):
                        o = acts.tile([128, Lq], F16,
                                      name=f"oT_{tag}_{h}_{c}",
                                      tag=f"oT_{2 * h + c}")
                        nc.vector.tensor_tensor(o[:], pos[2 * h + c],
                                                rrep[:], ALU.mult)
                        oT.append(o)
                return oT

            def out_proj_to_dram(tag, oT, w_dram, arins, arouts, Lq):
                """Out-proj in feature quarters (4-wide psum groups,
                Scalar staging copies), one AllReduce per HALF."""
                for q in range(4):
                    wt = wchunk(f"wo_{tag}_{q}", 4 * 1024)
                    nc.sync.dma_start(
                        wt[:], w_dram.ap()[:, 4096 * q:4096 * (q + 1)])
                    for sub in range(2):
                        ps = [pp(f"ps_op_{tag}_{q}_{sub}_{mi}", Lq)
                              for mi in range(4)]
                        for k in range(4):
                            for mi in range(4):
                                mm = 4 * sub + mi
                                nc.tensor.matmul(
                                    ps[mi],
                                    wt[:, 1024 * k + 128 * mm:
                                       1024 * k + 128 * (mm + 1)],
                                    oT[k][:],
                                    start=(k == 0), stop=(k == 3))
                        st = acts.tile([128, 4 * Lq], F16,
                                       name=f"st_{tag}_{q}_{sub}",
                                       tag="stage", bufs=3,
                                       padded_shape=[128, 4 * 512])
                        for mi in range(4):
                            nc.scalar.copy(
                                st[:, mi * Lq:(mi + 1) * Lq], ps[mi])
                        off = ((8 * (q % 2)) + 4 * sub) * Lq
                        nc.sync.dma_start(
                            arins[q // 2][:, off:off + 4 * Lq], st[:])
                    if q % 2 == 1:
                        nc.gpsimd.collective_compute(
                            "AllReduce", ALU.add, replica_groups=replica,
                            ins=[arins[q // 2].opt()],
                            outs=[arouts[q // 2].opt()])

            def mk_ar(tag, L, n=2):
                arins = [dram.tile([128, 16 * L], F16, name=f"{tag}i{q}",
                                   tag=f"{tag}i{q}") for q in range(n)]
                arouts = [dram.tile([128, 16 * L], F16, name=f"{tag}o{q}",
                                    tag=f"{tag}o{q}", addr_space="Shared")
                          for q in range(n)]
                return arins, arouts

            def stat_acc(tag, L, want_w=False):
                sacc = acts.tile([128, L], F16, name=f"sacc_{tag}",
                                 tag=f"sacc_{tag}")
                nc.vector.memset(sacc[:], 0.0)
                qacc = acts.tile([128, L], F32, name=f"qacc_{tag}",
                                 tag=f"qacc_{tag}")
                nc.vector.memset(qacc[:], 0.0)
                wacc = None
                if want_w:
                    wacc = acts.tile([128, L], F32, name=f"wacc_{tag}",
                                     tag=f"wacc_{tag}")
                    nc.vector.memset(wacc[:], 0.0)
                return sacc, qacc, wacc

            def resid_chase(tag, arouts, x_ts, L, sacc, qacc, wacc=None,
                            halves=range(2)):
                """Per AR half: DMA the landed data, add the residual
                in place (DVE), accumulate LN stats tile-wise on DVE:
                sacc += x, sq = x^2 (Scalar), qacc += sq, and optionally
                wacc += ws (.) x (Scalar per-partition scale + DVE)."""
                for hh in halves:
                    for s in range(4):
                        b = acts.tile([128, 4 * L], F16,
                                      name=f"arb_{tag}_{hh}_{s}", tag="arb",
                                      bufs=3, padded_shape=[128, 4 * 512])
                        nc.sync.dma_start(
                            b[:], arouts[hh][:, 4 * s * L:4 * (s + 1) * L])
                        g = 2 * hh + s // 2
                        off = (s % 2) * 4 * L
                        nc.vector.tensor_tensor(
                            x_ts[g][:, off:off + 4 * L], b[:],
                            x_ts[g][:, off:off + 4 * L], ALU.add)
                        for kk in range(4):
                            k = 16 * hh + 4 * s + kk
                            xk = xs(x_ts, L, k)
                            nc.vector.tensor_tensor(sacc[:], sacc[:], xk,
                                                    ALU.add)
                            sq = acts.tile([128, L], F16,
                                           name=f"sq_{tag}_{k}", tag="sqt",
                                           bufs=3, padded_shape=[128, 512])
                            nc.scalar.square(sq[:], xk)
                            nc.vector.tensor_tensor(qacc[:], qacc[:], sq[:],
                                                    ALU.add)
                            if wacc is not None:
                                wm = acts.tile([128, L], F16,
                                               name=f"wm_{tag}_{k}",
                                               tag="wmt", bufs=3,
                                               padded_shape=[128, 512])
                                nc.scalar.activation(wm[:], xk, AF.Copy,
                                                     scale=ws32[:, k:k + 1])
                                nc.vector.tensor_tensor(wacc[:], wacc[:],
                                                        wm[:], ALU.add)

            def ln_finalize(tag, s1p, s2p, L):
                mean = acts.tile([1, L], F32, name=f"mean_{tag}", tag="lmean")
                var = acts.tile([1, L], F32, name=f"var_{tag}", tag="lvar")
                tmpa = acts.tile([1, L], F32, name=f"tmpa_{tag}", tag="ltmp")
                r0 = acts.tile([1, L], F32, name=f"r0_{tag}", tag="lr0")
                nc.scalar.mul(mean[:], s1p, 1.0 / E)
                nc.scalar.mul(var[:], s2p, 1.0 / E)
                nc.scalar.square(tmpa[:], mean[:])
                nc.vector.tensor_sub(var[:], var[:], tmpa[:])
                nc.vector.tensor_scalar_add(var[:], var[:], 1e-5)
                nc.scalar.sqrt(tmpa[:], var[:])
                nc.vector.reciprocal(r0[:], tmpa[:])
                nc.vector.tensor_tensor(tmpa[:], r0[:], r0[:], ALU.mult)
                nc.vector.tensor_tensor(tmpa[:], tmpa[:], var[:], ALU.mult)
                nc.vector.tensor_scalar(tmpa[:], tmpa[:], -0.5, 1.5, ALU.mult,
                                        ALU.add)
                rstd = acts.tile([1, L], F16, name=f"rstd_{tag}", tag="rstd")
                nmr = acts.tile([1, L], F16, name=f"nmr_{tag}", tag="nmr")
                nc.vector.tensor_tensor(rstd[:], r0[:], tmpa[:], ALU.mult)
                nc.vector.scalar_tensor_tensor(nmr[:], mean[:], -1.0, rstd[:],
                                               ALU.mult, ALU.mult)
                Apsum = pb(f"ps_A_{tag}", L)
                nc.tensor.matmul(Apsum, ones_row[:], rstd[:], start=True,
                                 stop=True)
                Bpsum = pb(f"ps_B_{tag}", L)
                nc.tensor.matmul(Bpsum, ones_row[:], nmr[:], start=True,
                                 stop=True)
                Asb = acts.tile([128, L], F16, name=f"A_{tag}", tag="Asb")
                nc.scalar.copy(Asb[:], Apsum)
                Bsb = acts.tile([128, L], F16, name=f"B_{tag}", tag="Bsb")
                nc.scalar.copy(Bsb[:], Bpsum)
                return rstd, nmr, Asb, Bsb

            def stats_finalize(tag, sacc, qacc, L):
                q16 = acts.tile([128, L], F16, name=f"q16_{tag}",
                                tag=f"q16_{tag}")
                nc.scalar.copy(q16[:], qacc[:])
                s1p = pstat(f"ps_s1_{tag}", L)
                nc.tensor.matmul(s1p, ones_col[:], sacc[:], start=True,
                                 stop=True)
                s2p = pstat(f"ps_s2_{tag}", L)
                nc.tensor.matmul(s2p, ones_col[:], q16[:], start=True,
                                 stop=True)
                return ln_finalize(tag, s1p, s2p, L)

            def ln_apply(x_ts, L, Asb, Bsb, eng, dump=None):
                for k in range(ET):
                    eng.tensor_tensor(xs(x_ts, L, k), xs(x_ts, L, k),
                                      Asb[:], ALU.mult)
                    eng.tensor_tensor(xs(x_ts, L, k), xs(x_ts, L, k),
                                      Bsb[:], ALU.add)
                if dump is not None:
                    for g in range(ET // KG):
                        nc.sync.dma_start(
                            dump.ap()[:, KG * L * g:KG * L * (g + 1)],
                            x_ts[g][:])

            # ================= program =================
            cat_ts = load_x("catx", catp_d, ncat, ET // KG)

            # ---- MHA1 (cat self-attention) -> AR1 rides the empty CC ----
            q1 = proj_fm("q1", wd["q1"], cat_ts, ncat, "q")
            rem_ts = load_x("remx", remp_d, nrem, ET // KG)
            k1 = proj_fm("k1", wd["k1"], cat_ts, ncat, "k")
            v1 = proj_tm("v1", wd["v1"], cat_ts, ncat, JC)
            o1 = attention("a1", q1, k1, v1, ncat, ncat, JC)
            arin1, arout1 = mk_ar("ar1", ncat)
            out_proj_to_dram("op1", o1, wd["o1"], arin1, arout1, ncat)

            # ---- x1 residual + stats chase (engine ops only; the PE
            # finalize matmuls are emitted after op2 so the PE queue
            # never head-of-line blocks on AR1) ----
            sacc1, qacc1, _ = stat_acc("x1", ncat)
            resid_chase("x1", arout1, cat_ts, ncat, sacc1, qacc1)

            # ---- MHA2 (rem self-attention) -> AR2 quarters ----
            q2 = proj_fm("q2", wd["q2"], rem_ts, nrem, "q")
            k2 = proj_fm("k2", wd["k2"], rem_ts, nrem, "k")
            v2 = proj_tm("v2", wd["v2"], rem_ts, nrem, JR)
            o2 = attention("a2", q2, k2, v2, nrem, nrem, JR)
            arin2, arout2 = mk_ar("ar2", nrem)
            out_proj_to_dram("op2", o2, wd["o2"], arin2, arout2, nrem)

            # ---- x1 LN finalize + apply (DVE), then kc/vc ----
            rstd1, nmr1, A1sb, B1sb = stats_finalize("x1", sacc1, qacc1,
                                                     ncat)
            ln_apply(cat_ts, ncat, A1sb, B1sb, nc.vector,
                     dump=dbg.get("dbg_x1"))
            kc = proj_fm("kc", wd["kc"], cat_ts, ncat, "k")
            vc = proj_tm("vc", wd["vc"], cat_ts, ncat, JC)

            # ---- r residual-add + stats chase (DVE adds feed qc_u) ----
            saccr, qaccr, _ = stat_acc("r", nrem)
            resid_chase("r", arout2, rem_ts, nrem, saccr, qaccr)

            # ---- qc projects RAW rsum chasing AR2; LN commuted ----
            rstdr_f = [None]

            qc_ps = []
            chunks_qc = []
            for g in range(ET // KG):
                wt = wchunk(f"w_qc_{g}", KG * DLOC)
                nc.sync.dma_start(
                    wt[:],
                    wd["qc"].ap()[:, KG * DLOC * g:KG * DLOC * (g + 1)])
                chunks_qc.append(wt)
            qc_ps = [pp(f"ps_qc_{m}", nrem) for m in range(4)]
            for k in range(ET):
                g, kk = k // KG, k % KG
                for m in range(4):
                    nc.tensor.matmul(
                        qc_ps[m],
                        chunks_qc[g][:, kk * DLOC + 128 * m:
                                     kk * DLOC + 128 * (m + 1)],
                        xs(rem_ts, nrem, k),
                        start=(k == 0), stop=(k == ET - 1))
            # r LN stats finalize (PE matmuls sit right after qc_u chase)
            rstdr, nmrr, Arsb, Brsb = stats_finalize("r", saccr, qaccr,
                                                     nrem)
            qc = []
            for m in range(4):
                r1p = pp(f"ps_r1_qc_{m}", nrem)
                nc.tensor.matmul(r1p, wq1r_sb[:, 128 * m:128 * (m + 1)],
                                 nmrr[:], start=True, stop=True)
                o = acts.tile([128, nrem], F16, name=f"qc_{m}", tag=f"q_{m}")
                nc.vector.tensor_tensor(o[:], qc_ps[m], Arsb[:], ALU.mult)
                nc.vector.tensor_tensor(o[:], o[:], r1p, ALU.add)
                qc.append(o)

            # ---- MHAc (q from r-normed, kv from x1) ----
            oc = attention("ac", qc, kc, vc, nrem, ncat, JC)
            arinc, aroutc = mk_ar("arc", nrem)
            out_proj_to_dram("opc", oc, wd["oc"], arinc, aroutc, nrem)

            # ---- materialize r-normed in place (x2 residual base);
            # Pool engine so the DVE stays free for the x2 chase ----
            ln_apply(rem_ts, nrem, Arsb, Brsb, nc.vector,
                     dump=dbg.get("dbg_r"))

            # ---- x2 = LN(r + ARc) chase; f1 chases quarter-wise m0..5,
            # LN commuted into the f1 pre-activations ----
            sacc2, qacc2, wacc2 = stat_acc("x2", nrem, want_w=True)

            chunks_f1 = []
            for g in range(ET // 4):
                wt = wchunk(f"w_f1_{g}", 4 * 768)
                nc.sync.dma_start(
                    wt[:],
                    wd["f1a"].ap()[:, 4 * 768 * g:4 * 768 * (g + 1)])
                chunks_f1.append(wt)

            ps6 = [pp(f"ps_f1_{m}", nrem) for m in range(6)]
            for hh in range(2):
                resid_chase("x2", aroutc, rem_ts, nrem, sacc2, qacc2,
                            wacc=wacc2, halves=[hh])
                for kk in range(16):
                    k = 16 * hh + kk
                    for m in range(6):
                        nc.tensor.matmul(
                            ps6[m],
                            chunks_f1[k // 4][:, (k % 4) * 768 + 128 * m:
                                              (k % 4) * 768 + 128 * (m + 1)],
                            xs(rem_ts, nrem, k),
                            start=(k == 0), stop=(k == ET - 1))
            rstd2, nmr2, A2sb, B2sb = stats_finalize("x2", sacc2, qacc2,
                                                     nrem)

            def f1_correct(m, psrc):
                u = acts.tile([128, nrem], F16, name=f"f1u_{m}", tag="f1u",
                              bufs=2, padded_shape=[128, 512])
                nc.vector.tensor_tensor(u[:], psrc, A2sb[:], ALU.mult)
                opp = pp(f"ps_f1o_{m}", nrem)
                nc.tensor.matmul(opp, f1rs_sb[:, 128 * m:128 * (m + 1)],
                                 nmr2[:], start=True, stop=True)
                nc.vector.tensor_tensor(u[:], u[:], opp, ALU.add)
                h = acts.tile([128, nrem], F16, name=f"hT_{m}", tag=f"hT_{m}")
                nc.scalar.activation(h[:], u[:], AF.Gelu)
                return h

            hT = [None] * 8
            for m in range(6):
                hT[m] = f1_correct(m, ps6[m])
            # m6/m7: all data resident, run back-to-back
            chunks_f1b = []
            for g in range(2):
                wt = wchunk(f"w_f1b_{g}", 16 * 256)
                nc.sync.dma_start(
                    wt[:], wd["f1b"].ap()[:, 4096 * g:4096 * (g + 1)])
                chunks_f1b.append(wt)
            ps2 = [pp(f"ps_f1b_{m}", nrem) for m in range(2)]
            for k in range(ET):
                for i in range(2):
                    nc.tensor.matmul(
                        ps2[i],
                        chunks_f1b[k // 16][:, (k % 16) * 256 + 128 * i:
                                            (k % 16) * 256 + 128 * (i + 1)],
                        xs(rem_ts, nrem, k),
                        start=(k == 0), stop=(k == ET - 1))
            for i, m in enumerate((6, 7)):
                hT[m] = f1_correct(m, ps2[i])

            # ---- wx2 = Ws . x2sum from the Pool accumulator ----
            w16 = acts.tile([128, nrem], F16, name="w16", tag="w16")
            nc.scalar.copy(w16[:], wacc2[:])
            wxu = pstat("ps_wx2", nrem)
            nc.tensor.matmul(wxu, ones_col[:], w16[:], start=True, stop=True)
            wx2 = acts.tile([1, nrem], F32, name="wx2", tag="wx2")
            nc.vector.tensor_tensor(wx2[:], wxu, rstd2[:], ALU.mult)
            nc.vector.scalar_tensor_tensor(wx2[:], nmr2[:],
                                           consts[0:1, 0:1], wx2[:],
                                           ALU.mult, ALU.add)

            # ---- linear logit stats from hT: s1 = c2.g ; ws += w2s.g/256 --
            c2p = pstat("ps_c2", nrem)
            for m in range(8):
                nc.tensor.matmul(c2p, c2w_sb[:, m:m + 1], hT[m][:],
                                 start=(m == 0), stop=(m == 7))
            w2p = pstat("ps_w2s", nrem)
            for m in range(8):
                nc.tensor.matmul(w2p, c2w_sb[:, 8 + m:9 + m], hT[m][:],
                                 start=(m == 0), stop=(m == 7))
            s1part = acts.tile([1, 512], F32, name="s1part", tag="s1part")
            wspart = acts.tile([1, 512], F32, name="wspart", tag="wspart")
            nc.vector.memset(s1part[:], 1.0)
            nc.vector.memset(wspart[:], 0.0)
            nc.vector.tensor_copy(s1part[:, 0:nrem], c2p)
            nc.vector.tensor_scalar(wx2[:], wx2[:], 1.0 / NCORES, 0.0,
                                    ALU.mult, ALU.add)
            nc.vector.scalar_tensor_tensor(wspart[:, 0:nrem], w2p,
                                           1.0 / 256.0, wx2[:],
                                           ALU.mult, ALU.add)
            arin6 = dram.tile([128, 16], F32, name="arin6", tag="arin6")
            arout6 = dram.tile([128, 16], F32, name="arout6", tag="arout6",
                               addr_space="Shared")
            nc.sync.dma_start(arin6[:, 0:4], s1part[:])
            nc.sync.dma_start(arin6[:, 4:8], wspart[:])
            nc.sync.dma_start(arin6[:, 12:16], s1part[:])

            # ---- materialize x2 in place for the f2 residual fold ----
            ln_apply(rem_ts, nrem, A2sb, B2sb, nc.vector,
                     dump=dbg.get("dbg_x2"))

            # ---- FFN f2: partial = x2/8 + Wf2_shard^T hT; RS/quarter ----
            arin4 = [dram.tile([128, 16 * nrem], F16, name=f"ar4i{q}",
                               tag=f"ar4i{q}") for q in range(2)]
            rs4 = [dram.tile([16, 16 * nrem], F16, name=f"rs4{q}",
                             tag=f"rs4{q}") for q in range(2)]
            for q in range(4):
                for sub in range(2):
                    wt = wchunk(f"w_f2_{q}_{sub}", 8 * 512)
                    nc.sync.dma_start(
                        wt[:], wd["f2"].ap()[:, 4096 * (2 * q + sub):
                                             4096 * (2 * q + sub + 1)])
                    ps = [pp(f"ps_f2_{q}_{sub}_{mi}", nrem)
                          for mi in range(4)]
                    for k in range(8):
                        for mi in range(4):
                            nc.tensor.matmul(
                                ps[mi],
                                wt[:, 512 * k + 128 * mi:
                                   512 * k + 128 * (mi + 1)],
                                hT[k][:],
                                start=(k == 0), stop=(k == 7))
                    st = acts.tile([128, 4 * nrem], F16,
                                   name=f"st_f2_{q}_{sub}",
                                   tag="stage", bufs=3,
                                   padded_shape=[128, 4 * 512])
                    for mi in range(4):
                        mm = 4 * sub + mi
                        m = 8 * q + mm
                        nc.vector.scalar_tensor_tensor(
                            st[:, mi * nrem:(mi + 1) * nrem],
                            xs(rem_ts, nrem, m), 1.0 / NCORES, ps[mi],
                            ALU.mult, ALU.add)
                    off = ((8 * (q % 2)) + 4 * sub) * nrem
                    nc.sync.dma_start(
                        arin4[q // 2][:, off:off + 4 * nrem], st[:])
                if q % 2 == 1:
                    nc.gpsimd.collective_compute(
                        "ReduceScatter", ALU.add, replica_groups=replica,
                        ins=[arin4[q // 2].opt()],
                        outs=[rs4[q // 2].opt()])

            # ---- s2 from the scattered summed features, chased/half ----
            CW = 4 * nrem  # columns per rs4 read chunk (4 chunks/half)
            s2p = pstat("ps_rs2", nrem)
            for cch in range(8):
                q, hcol = cch // 4, (cch % 4) * CW
                bt = acts.tile([16, CW], F16, name=f"rsb_{cch}", tag="rsb",
                               bufs=2, padded_shape=[16, 4 * 512])
                nc.sync.dma_start(bt[:], rs4[q][:, hcol:hcol + CW])
                sq = acts.tile([16, CW], F16, name=f"rssq_{cch}", tag="rssq",
                               bufs=2, padded_shape=[16, 4 * 512])
                nc.scalar.square(sq[:], bt[:])
                for s in range(4):
                    k = cch * 4 + s
                    nc.tensor.matmul(s2p, ones_col[0:16, :],
                                     sq[:, s * nrem:(s + 1) * nrem],
                                     start=(k == 0), stop=(k == ET - 1))
            s2part = acts.tile([1, 512], F32, name="s2part", tag="s2part")
            nc.vector.memset(s2part[:], 1.0)
            nc.vector.tensor_copy(s2part[:, 0:nrem], s2p)
            nc.sync.dma_start(arin6[:, 8:12], s2part[:])
            nc.gpsimd.collective_compute(
                "AllReduce", ALU.add, replica_groups=replica,
                ins=[arin6.opt()], outs=[arout6.opt()])

            # ---- final logit: affine-LN identity, 128-wide blocked ----
            gsb = acts.tile([128, 12], F32, name="gsb", tag="gsb")
            nc.sync.dma_start(gsb[:], arout6[:, 0:12])
            g1, g2, g3 = gsb[:, 0:4], gsb[:, 4:8], gsb[:, 8:12]
            mean = acts.tile([128, 4], F32, name="mean_l", tag="lmean_l")
            var = acts.tile([128, 4], F32, name="var_l", tag="lvar_l")
            tmpa = acts.tile([128, 4], F32, name="tmpa_l", tag="ltmp_l")
            r0 = acts.tile([128, 4], F32, name="r0_l", tag="lr0_l")
            nc.scalar.mul(mean[:], g1, 1.0 / E)
            nc.scalar.mul(var[:], g3, 1.0 / E)
            nc.scalar.square(tmpa[:], mean[:])
            nc.vector.tensor_sub(var[:], var[:], tmpa[:])
            nc.vector.tensor_scalar_add(var[:], var[:], 1e-5)
            nc.scalar.sqrt(tmpa[:], var[:])
            nc.vector.reciprocal(r0[:], tmpa[:])
            nc.vector.tensor_tensor(tmpa[:], r0[:], r0[:], ALU.mult)
            nc.vector.tensor_tensor(tmpa[:], tmpa[:], var[:], ALU.mult)
            nc.vector.tensor_scalar(tmpa[:], tmpa[:], -0.5, 1.5,
                                    ALU.mult, ALU.add)
            rstd = acts.tile([128, 4], F32, name="rstd_l", tag="rstd_l")
            nc.vector.tensor_tensor(rstd[:], r0[:], tmpa[:], ALU.mult)
            nmr = acts.tile([128, 4], F32, name="nmr_l", tag="nmr_l")
            nc.vector.scalar_tensor_tensor(nmr[:], mean[:], -1.0,
                                           rstd[:], ALU.mult, ALU.mult)
            wdot = acts.tile([128, 4], F32, name="wdot", tag="wdot")
            nc.vector.tensor_tensor(wdot[:], rstd[:], g2, ALU.mult)
            lsb = acts.tile([128, 4], F32, name="lsb", tag="lsb")
            nc.vector.scalar_tensor_tensor(lsb[:], nmr[:],
                                           consts[:, 0:1], wdot[:],
                                           ALU.mult, ALU.add)
            nc.sync.dma_start(logits_d.ap(), lsb[:])

    nc.compile()
    return nc


# ----------------------------------------------------------------------------
# host orchestration
# ----------------------------------------------------------------------------

def _packx(XT):
    """[E, L] fp32 -> [128, ET*L] fp16 feature-block pack."""
    L = XT.shape[1]
    return np.ascontiguousarray(
        XT.reshape(ET, 128, L).transpose(1, 0, 2).reshape(128, ET * L)
        .astype(np.float16))


def _prep_in_maps(vision_feature, text_embed, sel_idx, rem_idx,
                  Wqkv1, Wo1, Wqkv2, Wo2, Wqkvc, Woc, Wf1, Wf2, Ws):
    f16 = np.float16
    sel = vision_feature[sel_idx]
    rem = vision_feature[rem_idx]
    cat = np.concatenate([sel, text_embed], axis=0)

    remp = _packx(np.ascontiguousarray(rem.T))
    catp = _packx(np.ascontiguousarray(cat.T))
    ws_pack = np.ascontiguousarray(Ws[0].reshape(ET, 128).T.astype(f16))
    consts = np.broadcast_to(
        np.array([[np.float64(Ws.astype(np.float64).sum()), 0.0]],
                 np.float32), (128, 2)).copy()

    in_maps = []
    for c in range(NCORES):
        hs = slice(DLOC * c, DLOC * (c + 1))
        fs = slice(FLOC * c, FLOC * (c + 1))
        m = {"remp": remp, "catp": catp, "wsp": ws_pack, "consts": consts}
        for l, Wqkv, Wo in (("1", Wqkv1, Wo1), ("2", Wqkv2, Wo2),
                            ("c", Wqkvc, Woc)):
            Wq, Wk, Wv = Wqkv[:E], Wqkv[E:2 * E], Wqkv[2 * E:]
            for nm, W in (("q", Wq), ("k", Wk), ("v", Wv)):
                A = W[hs].T  # [E, DLOC]
                m[f"w{nm}{l}"] = np.ascontiguousarray(
                    A.reshape(ET, 128, DLOC).transpose(1, 0, 2)
                    .reshape(128, ET * DLOC).astype(f16))
            WoT = Wo[:, hs].T  # [DLOC, E]
            m[f"wo{l}"] = np.ascontiguousarray(
                WoT.reshape(4, 128, 4, 1024).transpose(1, 2, 0, 3)
                .reshape(128, 4 * E).astype(f16))
        # (Wq_c^T 1) for the commuted-LN fixup of qc
        m["wq1r"] = np.ascontiguousarray(
            Wqkvc[:E][hs].astype(np.float64).sum(axis=1).reshape(1, DLOC)
            .astype(f16))
        A = Wf1[fs].T  # [E, FLOC]
        m["wf1a"] = np.ascontiguousarray(
            A[:, 0:768].reshape(ET, 128, 768).transpose(1, 0, 2)
            .reshape(128, ET * 768).astype(f16))
        m["wf1b"] = np.ascontiguousarray(
            A[:, 768:1024].reshape(ET, 128, 256).transpose(1, 0, 2)
            .reshape(128, ET * 256).astype(f16))
        W2T = Wf2[:, fs].T  # [FLOC, E]
        m["wf2"] = np.ascontiguousarray(
            W2T.reshape(8, 128, 8, 512).transpose(1, 2, 0, 3)
            .reshape(128, 8 * E).astype(f16))
        c2 = Wf2[:, fs].astype(np.float64).sum(axis=0)  # [FLOC]
        w2s = 256.0 * (Ws[0].astype(np.float64) @ Wf2[:, fs].astype(np.float64))
        c2w = np.concatenate([c2.reshape(8, 128).T, w2s.reshape(8, 128).T],
                             axis=1)  # [128, 16]
        m["c2w"] = np.ascontiguousarray(c2w.astype(f16))
        m["f1rs"] = np.ascontiguousarray(
            Wf1[fs].astype(np.float64).sum(axis=1).reshape(1, FLOC)
            .astype(f16))
        in_maps.append(m)
    return in_maps


def run_device(in_maps, ncat_real, nrem_real, dumps=False, trace=False):
    from concourse.bass_utils import run_bass_kernel_spmd

    key = (ncat_real, nrem_real, dumps)
    if key not in _CACHE:
        _CACHE[key] = _build_device(ncat_real, nrem_real, dumps=dumps)
    nc = _CACHE[key]
    return run_bass_kernel_spmd(nc, in_maps, list(range(NCORES)), trace=trace)


def _kernel_impl(inputs, debug=False, trace=False):
    vision_feature = np.asarray(inputs["vision_feature"], np.float32)
    text_embed = np.asarray(inputs["text_embed"], np.float32)
    attention_mask = np.asarray(inputs["attention_mask"])

    biases_zero = all(
        not np.any(np.asarray(inputs[b]))
        for b in ("bqkv1", "bo1", "bqkv2", "bo2", "bqkvc", "boc",
                  "bf1", "bf2", "bs"))
    if (not bool(attention_mask.all())) or (not biases_zero):
        return (_reference_np(**{k: np.asarray(v) for k, v in inputs.items()}),
                None)

    t, sel_idx, rem_idx = _score_partition(vision_feature, text_embed,
                                           attention_mask)
    ncat_real = t + text_embed.shape[0]
    nrem_real = vision_feature.shape[0] - t
    kk = int(t * EXPAND)

    in_maps = _prep_in_maps(
        vision_feature, text_embed, sel_idx, rem_idx,
        np.asarray(inputs["Wqkv1"], np.float32),
        np.asarray(inputs["Wo1"], np.float32),
        np.asarray(inputs["Wqkv2"], np.float32),
        np.asarray(inputs["Wo2"], np.float32),
        np.asarray(inputs["Wqkvc"], np.float32),
        np.asarray(inputs["Woc"], np.float32),
        np.asarray(inputs["Wf1"], np.float32),
        np.asarray(inputs["Wf2"], np.float32),
        np.asarray(inputs["Ws"], np.float32))
    res = run_device(in_maps, ncat_real, nrem_real, dumps=debug, trace=trace)
    logits = res.results[0]["logits"][0, :nrem_real]
    es = (1.0 / (1.0 + np.exp(-logits.astype(np.float32))))
    ei = np.argsort(-es, kind="stable")[:kk]
    final = np.sort(np.concatenate([sel_idx, rem_idx[ei]]))
    return vision_feature[final], res


def kernel(**inputs):
    out, _ = _kernel_impl(inputs)
    return out


# revision 3
# speedup vs baseline: 1.0257x; 1.0200x over previous
"""Trainium2 Bass kernel for nn_CosSimRouter_learn_49778670960796. v2.

Schedule-restructured vs v1:
  * All big collectives quartered (one AR per out-proj feature quarter)
    and fired as soon as each quarter's staging lands; every consumer
    chases quarter-wise.
  * Program order: MHA1(cat) first (AR1 rides the empty early CC
    window), then MHA2(rem), then kc/vc, then qc which projects the RAW
    rem residual (r pre-LN) chasing AR2 quarters; the per-token LN
    scale/shift is commuted through the linear projection and applied
    as qc = rstd (x) qc_u + (Wq^T 1) (x) nmr afterwards (rank-1 via
    matmul + DVE).
  * LN statistics are accumulated on DVE/Pool/Scalar tile-wise while
    quarters land (no 32x ones-matmul chains on the PE): sacc (Pool,
    fp16), sq (Scalar) + qacc (DVE, fp32), then a single ones-matmul
    partition-reduce each.  Ws.x2sum (wx2) likewise via Scalar
    per-partition-scale copy + Pool fp32 accumulate.
  * f1 chases ARc quarters 6 psum banks wide (m0-5), m6/m7 run
    back-to-back after the last quarter; PSUM ring split 6 ("pp") + 2
    ("pps" for [1,L] stats / LN broadcast psums).
  * f2 fires one ReduceScatter per output-feature quarter; the s2
    stat chase + tiny fp32 AR (s1/ws/s2) close the kernel.
"""

import numpy as np

E = 4096
H = 16
HID = 8192
GAMMA = 0.2
TEMP = 0.05
EXPAND = 0.7
NCORES = 8
ET = E // 128  # 32 feature tiles
DH = E // H  # 256
HL = H // NCORES  # 2 heads per core
DLOC = HL * DH  # 512 local head dims
FLOC = HID // NCORES  # 1024 local ffn hidden
KG = 8  # k-blocks per weight/act chunk (== one AR quarter)

_CACHE = {}


# ----------------------------------------------------------------------------
# host-side reference math (numpy, fp32) for the scoring stage + fallback
# ----------------------------------------------------------------------------

def _score_partition(vision_feature, text_embed, attention_mask):
    vf = vision_feature.astype(np.float32)
    te = text_embed.astype(np.float32)
    vn = vf / np.maximum(np.linalg.norm(vf, axis=-1, keepdims=True), 1e-8)
    tn = te / np.maximum(np.linalg.norm(te, axis=-1, keepdims=True), 1e-8)
    cs = vn @ tn.T
    cs = np.where(attention_mask[None, :], cs, np.float32(0.0))
    m = cs.max(axis=-1) / np.float32(TEMP)
    e = np.exp(m - m.max())
    scores = e / e.sum()
    order = np.argsort(-scores, kind="stable")
    cum = np.cumsum(scores[order])
    t = int((cum <= GAMMA).sum())
    return t, order[:t], order[t:]


def _ln_np(x):
    m = x.mean(-1, keepdims=True)
    v = ((x - m) ** 2).mean(-1, keepdims=True)
    return (x - m) / np.sqrt(v + 1e-5)


def _gelu_np(x):
    import math

    erf = np.frompyfunc(math.erf, 1, 1)
    return (x * 0.5 * (1.0 + erf(x / math.sqrt(2.0)).astype(np.float64))
            ).astype(x.dtype)


def _mha_np(q_in, kv_in, Wqkv, bqkv, Wo, bo):
    dh = E // H
    Wq, Wk, Wv = np.split(Wqkv, 3, axis=0)
    bq, bk, bv = np.split(bqkv, 3)
    q = (q_in @ Wq.T + bq).reshape(-1, H, dh)
    k = (kv_in @ Wk.T + bk).reshape(-1, H, dh)
    v = (kv_in @ Wv.T + bv).reshape(-1, H, dh)
    att = np.einsum("qhd,khd->hqk", q, k) / np.float32(np.sqrt(dh))
    att = att - att.max(-1, keepdims=True)
    att = np.exp(att)
    att /= att.sum(-1, keepdims=True)
    o = np.einsum("hqk,khd->qhd", att.astype(np.float32), v).reshape(-1, E)
    return o @ Wo.T + bo


def _reference_np(vision_feature, text_embed, attention_mask,
                  Wqkv1, bqkv1, Wo1, bo1, Wqkv2, bqkv2, Wo2, bo2,
                  Wqkvc, bqkvc, Woc, boc, Wf1, bf1, Wf2, bf2, Ws, bs):
    t, sel_idx, rem_idx = _score_partition(vision_feature, text_embed,
                                           attention_mask)
    sel = vision_feature[sel_idx]
    rem = vision_feature[rem_idx]
    cat = np.concatenate([sel, text_embed], axis=0)
    x = _ln_np(_mha_np(cat, cat, Wqkv1, bqkv1, Wo1, bo1) + cat)
    r = _ln_np(_mha_np(rem, rem, Wqkv2, bqkv2, Wo2, bo2) + rem)
    x = _ln_np(_mha_np(r, x, Wqkvc, bqkvc, Woc, boc) + r)
    ffn = _gelu_np(x @ Wf1.T + bf1) @ Wf2.T + bf2
    x = _ln_np(x + ffn)
    logits = (x @ Ws.T + bs).squeeze(-1)
    es = 1.0 / (1.0 + np.exp(-logits))
    k = int(t * EXPAND)
    ei = np.argsort(-es, kind="stable")[:k]
    final = np.sort(np.concatenate([sel_idx, rem_idx[ei]]))
    return vision_feature[final]


# ----------------------------------------------------------------------------
# device program
# ----------------------------------------------------------------------------

def _build_device(ncat, nrem, dumps=False):
    import concourse.bacc as bacc
    import concourse.mybir as mybir
    import concourse.tile as tile

    dt = mybir.dt
    F32 = dt.float32
    F16 = dt.float16
    AF = mybir.ActivationFunctionType
    ALU = mybir.AluOpType

    JC = (ncat + 127) // 128  # kv partition tiles for cat (2)
    JR = (nrem + 127) // 128  # kv partition tiles for rem (4)

    nc = bacc.Bacc("TRN2", target_bir_lowering=False, debug=False,
                   num_devices=NCORES)

    # ---------------- DRAM I/O (all host-packed, see _prep_in_maps) --------
    remp_d = nc.dram_tensor("remp", [128, ET * nrem], F16, kind="ExternalInput")
    catp_d = nc.dram_tensor("catp", [128, ET * ncat], F16, kind="ExternalInput")
    wd = {}
    for l in ("1", "2", "c"):
        for p in ("q", "k", "v"):
            wd[p + l] = nc.dram_tensor(f"w{p}{l}", [128, ET * DLOC], F16,
                                       kind="ExternalInput")
        wd["o" + l] = nc.dram_tensor(f"wo{l}", [128, (DLOC // 128) * E], F16,
                                     kind="ExternalInput")
    wd["f1a"] = nc.dram_tensor("wf1a", [128, ET * 768], F16,
                               kind="ExternalInput")
    wd["f1b"] = nc.dram_tensor("wf1b", [128, ET * 256], F16,
                               kind="ExternalInput")
    wd["f2"] = nc.dram_tensor("wf2", [128, (FLOC // 128) * E], F16,
                              kind="ExternalInput")
    ws_d = nc.dram_tensor("wsp", [128, ET], F16, kind="ExternalInput")
    c2w_d = nc.dram_tensor("c2w", [128, 2 * (FLOC // 128)], F16,
                           kind="ExternalInput")
    f1rs_d = nc.dram_tensor("f1rs", [1, FLOC], F16, kind="ExternalInput")
    wq1r_d = nc.dram_tensor("wq1r", [1, DLOC], F16, kind="ExternalInput")
    consts_d = nc.dram_tensor("consts", [128, 2], F32, kind="ExternalInput")
    logits_d = nc.dram_tensor("logits", [1, 512], F32, kind="ExternalOutput")
    dbg = {}
    if dumps:
        for nm, L in (("dbg_x1", ncat), ("dbg_r", nrem), ("dbg_x2", nrem)):
            dbg[nm] = nc.dram_tensor(nm, [128, ET * L], F16,
                                     kind="ExternalOutput")

    replica = [list(range(NCORES))]

    with tile.TileContext(nc, num_cores=NCORES) as tc:
        with (
            tc.tile_pool(name="acts", bufs=1) as acts,
            tc.tile_pool(name="psum", bufs=1, space="PSUM") as psum,
            tc.tile_pool(name="dram", bufs=1, space="DRAM") as dram,
        ):
            # ---- constants ----
            ones_col = acts.tile([128, 1], F16, name="ones_col",
                                 tag="ones_col")
            nc.vector.memset(ones_col[:], 1.0)
            ones_row = acts.tile([1, 128], F16, name="ones_row",
                                 tag="ones_row")
            nc.vector.memset(ones_row[:], 1.0)
            ws_sb = acts.tile([128, ET], F16, name="ws_sb", tag="ws_sb")
            nc.sync.dma_start(ws_sb[:], ws_d.ap())
            ws32 = acts.tile([128, ET], F32, name="ws32", tag="ws32")
            nc.vector.tensor_copy(ws32[:], ws_sb[:])
            c2w_sb = acts.tile([128, 2 * (FLOC // 128)], F16, name="c2w_sb",
                               tag="c2w_sb")
            nc.sync.dma_start(c2w_sb[:], c2w_d.ap())
            consts = acts.tile([128, 2], F32, name="consts", tag="consts")
            nc.sync.dma_start(consts[:], consts_d.ap())
            f1rs_sb = acts.tile([1, FLOC], F16, name="f1rs_sb", tag="f1rs")
            nc.sync.dma_start(f1rs_sb[:], f1rs_d.ap())
            wq1r_sb = acts.tile([1, DLOC], F16, name="wq1r_sb", tag="wq1r")
            nc.sync.dma_start(wq1r_sb[:], wq1r_d.ap())

            # PSUM ring: 6 full banks ("pp") + 2 banks for [1,L] stats and
            # LN broadcast psums ("pps"). 6*2048 + 2*2048 = 16 KiB.
            def pp(name, L, parts=128):
                t_ = psum.tile([128, L], F32, name=name, tag="pp", bufs=6,
                               padded_shape=[128, 512])
                return t_[0:parts, :] if parts < 128 else t_[:]

            def pstat(name, L):
                return psum.tile([1, L], F32, name=name, tag="pps", bufs=1,
                                 padded_shape=[1, 512])[:]

            def pb(name, L):
                return psum.tile([128, L], F32, name=name, tag="ppb",
                                 bufs=1, padded_shape=[128, 512])[:]

            def wchunk(name, cols):
                return acts.tile([128, cols], F16, name=name, tag="wt",
                                 bufs=3, padded_shape=[128, 4096])

            # ---- activations: group tiles + slice helper ----
            def load_x(name, dram_t, L, ngroups):
                ts = []
                for g in range(ngroups):
                    xt = acts.tile([128, KG * L], F16, name=f"{name}_{g}",
                                   tag=f"{name}_{g}")
                    nc.sync.dma_start(
                        xt[:], dram_t.ap()[:, KG * L * g:KG * L * (g + 1)])
                    ts.append(xt)
                return ts

            def xs(ts, L, k):
                g, kk = k // KG, k % KG
                return ts[g][:, kk * L:(kk + 1) * L]

            # ---------------- building blocks ----------------
            def proj_fm(tagbase, w_dram, x_ts, L, outtag, correct=None):
                """q/k projection -> 4 tiles [128, L] fp16 (DLOC, L) layout.

                correct=(Asb, nmr_row, w1r_sb): instead of a plain PSUM
                copy, apply the commuted-LN fixup
                out_m = Asb (x) psum_m + (w1r_m (x) nmr).
                """
                chunks = []
                for g in range(ET // KG):
                    wt = wchunk(f"w_{tagbase}_{g}", KG * DLOC)
                    nc.sync.dma_start(
                        wt[:],
                        w_dram.ap()[:, KG * DLOC * g:KG * DLOC * (g + 1)])
                    chunks.append(wt)
                ps = [pp(f"ps_{tagbase}_{m}", L) for m in range(4)]
                for k in range(ET):
                    g, kk = k // KG, k % KG
                    for m in range(4):
                        nc.tensor.matmul(
                            ps[m],
                            chunks[g][:, kk * DLOC + 128 * m:
                                      kk * DLOC + 128 * (m + 1)],
                            xs(x_ts, L, k),
                            start=(k == 0), stop=(k == ET - 1))
                outs = []
                if correct is None:
                    for m in range(4):
                        o = acts.tile([128, L], F16, name=f"{tagbase}_{m}",
                                      tag=f"{outtag}_{m}")
                        nc.scalar.copy(o[:], ps[m])
                        outs.append(o)
                else:
                    Asb, nmr_row, w1r = correct
                    for m in range(4):
                        r1p = pp(f"ps_r1_{tagbase}_{m}", L)
                        nc.tensor.matmul(r1p,
                                         w1r[:, 128 * m:128 * (m + 1)],
                                         nmr_row[:], start=True, stop=True)
                        o = acts.tile([128, L], F16, name=f"{tagbase}_{m}",
                                      tag=f"{outtag}_{m}")
                        nc.vector.tensor_tensor(o[:], ps[m], Asb[:], ALU.mult)
                        nc.vector.tensor_tensor(o[:], o[:], r1p, ALU.add)
                        outs.append(o)
                return outs

            def proj_tm(tagbase, w_dram, x_ts, L, JT, vtag="v"):
                """v projection -> JT tiles [128, DLOC] fp16 (kv, DLOC)."""
                chunks = []
                for g in range(ET // KG):
                    wt = wchunk(f"w_{tagbase}_{g}", KG * DLOC)
                    nc.sync.dma_start(
                        wt[:],
                        w_dram.ap()[:, KG * DLOC * g:KG * DLOC * (g + 1)])
                    chunks.append(wt)
                ps = []
                for j in range(JT):
                    pj = min(128, L - 128 * j)
                    ps.append(pp(f"ps_{tagbase}_{j}", DLOC, parts=pj))
                for k in range(ET):
                    g, kk = k // KG, k % KG
                    for j in range(JT):
                        pj = min(128, L - 128 * j)
                        nc.tensor.matmul(
                            ps[j],
                            xs(x_ts, L, k)[:, 128 * j:128 * j + pj],
                            chunks[g][:, kk * DLOC:(kk + 1) * DLOC],
                            start=(k == 0), stop=(k == ET - 1))
                outs = []
                for j in range(JT):
                    pj = min(128, L - 128 * j)
                    o = acts.tile([128, DLOC], F16, name=f"{tagbase}_{j}",
                                  tag=f"{vtag}_{j}")
                    nc.scalar.copy(o[0:pj, :], ps[j])
                    outs.append(o)
                return outs

            def attention(tag, qT, kT, vT, Lq, Lkv, JT):
                exps_h = []
                for h in range(HL):
                    exps = []
                    for j in range(JT):
                        pj = min(128, Lkv - 128 * j)
                        p = pp(f"ps_s_{tag}_{h}_{j}", Lq, parts=pj)
                        for c in range(2):
                            nc.tensor.matmul(
                                p,
                                kT[2 * h + c][:, 128 * j:128 * j + pj],
                                qT[2 * h + c][:],
                                start=(c == 0), stop=(c == 1))
                        e = acts.tile([128, Lq], F16,
                                      name=f"es_{tag}_{h}_{j}",
                                      tag=f"expS_{h}_{j}")
                        nc.scalar.activation(e[0:pj, :], p, AF.Exp,
                                             scale=float(1.0 / np.sqrt(DH)))
                        exps.append(e)
                    exps_h.append(exps)
                rec2s = []
                for h in range(HL):
                    dsum = pstat(f"ps_d_{tag}_{h}", Lq)
                    for j in range(JT):
                        pj = min(128, Lkv - 128 * j)
                        nc.tensor.matmul(dsum, ones_col[0:pj, :],
                                         exps_h[h][j][0:pj, :],
                                         start=(j == 0), stop=(j == JT - 1))
                    den = acts.tile([1, Lq], F32, name=f"den_{tag}_{h}",
                                    tag="aden")
                    rec = acts.tile([1, Lq], F32, name=f"rec_{tag}_{h}",
                                    tag="arec")
                    nc.vector.tensor_copy(den[:], dsum)
                    nc.vector.reciprocal(rec[:], den[:])
                    nc.vector.tensor_tensor(den[:], den[:], rec[:], ALU.mult)
                    nc.vector.tensor_scalar(den[:], den[:], -1.0, 2.0,
                                            ALU.mult, ALU.add)
                    rec2 = acts.tile([1, Lq], F16, name=f"rec2_{tag}_{h}",
                                     tag=f"rec2_{h}")
                    nc.vector.tensor_tensor(rec2[:], rec[:], den[:], ALU.mult)
                    rec2s.append(rec2)
                pos = []
                for h in range(HL):
                    for c in range(2):
                        po = pp(f"ps_o_{tag}_{h}_{c}", Lq)
                        for j in range(JT):
                            pj = min(128, Lkv - 128 * j)
                            nc.tensor.matmul(
                                po,
                                vT[j][0:pj, 256 * h + 128 * c:
                                      256 * h + 128 * (c + 1)],
                                exps_h[h][j][0:pj, :],
                                start=(j == 0), stop=(j == JT - 1))
                        pos.append(po)
                oT = []
                for h in range(HL):
                    rrep_p = pp(f"ps_rr_{tag}_{h}", Lq)
                    nc.tensor.matmul(rrep_p, ones_row[:], rec2s[h][:],
                                     start=True, stop=True)
                    rrep = acts.tile([128, Lq], F32, name=f"rr_{tag}_{h}",
                                     tag=f"rrep_{h}")
                    nc.scalar.copy(rrep[:], rrep_p)
                    for c in range(2):
                        o = acts.tile([128, Lq], F16,
                                      name=f"oT_{tag}_{h}_{c}",
                                      tag=f"oT_{2 * h + c}")
                        nc.vector.tensor_tensor(o[:], pos[2 * h + c],
                                                rrep[:], ALU.mult)
                        oT.append(o)
                return oT

            def out_proj_to_dram(tag, oT, w_dram, arins, arouts, Lq):
                """Out-proj in feature quarters (4-wide psum groups,
                Scalar staging copies), one AllReduce per HALF."""
                for q in range(4):
                    wt = wchunk(f"wo_{tag}_{q}", 4 * 1024)
                    nc.sync.dma_start(
                        wt[:], w_dram.ap()[:, 4096 * q:4096 * (q + 1)])
                    for sub in range(2):
                        ps = [pp(f"ps_op_{tag}_{q}_{sub}_{mi}", Lq)
                              for mi in range(4)]
                        for k in range(4):
                            for mi in range(4):
                                mm = 4 * sub + mi
                                nc.tensor.matmul(
                                    ps[mi],
                                    wt[:, 1024 * k + 128 * mm:
                                       1024 * k + 128 * (mm + 1)],
                                    oT[k][:],
                                    start=(k == 0), stop=(k == 3))
                        st = acts.tile([128, 4 * Lq], F16,
                                       name=f"st_{tag}_{q}_{sub}",
                                       tag="stage", bufs=3,
                                       padded_shape=[128, 4 * 512])
                        for mi in range(4):
                            nc.scalar.copy(
                                st[:, mi * Lq:(mi + 1) * Lq], ps[mi])
                        off = ((8 * (q % 2)) + 4 * sub) * Lq
                        nc.sync.dma_start(
                            arins[q // 2][:, off:off + 4 * Lq], st[:])
                    if q % 2 == 1:
                        nc.gpsimd.collective_compute(
                            "AllReduce", ALU.add, replica_groups=replica,
                            ins=[arins[q // 2].opt()],
                            outs=[arouts[q // 2].opt()])

            def mk_ar(tag, L, n=2):
                arins = [dram.tile([128, 16 * L], F16, name=f"{tag}i{q}",
                                   tag=f"{tag}i{q}") for q in range(n)]
                arouts = [dram.tile([128, 16 * L], F16, name=f"{tag}o{q}",
                                    tag=f"{tag}o{q}", addr_space="Shared")
                          for q in range(n)]
                return arins, arouts

            def stat_acc(tag, L, want_w=False):
                sacc = acts.tile([128, L], F16, name=f"sacc_{tag}",
                                 tag=f"sacc_{tag}")
                nc.vector.memset(sacc[:], 0.0)
                qacc = acts.tile([128, L], F32, name=f"qacc_{tag}",
                                 tag=f"qacc_{tag}")
                nc.vector.memset(qacc[:], 0.0)
                wacc = None
                if want_w:
                    wacc = acts.tile([128, L], F32, name=f"wacc_{tag}",
                                     tag=f"wacc_{tag}")
                    nc.vector.memset(wacc[:], 0.0)
                return sacc, qacc, wacc

            def resid_chase(tag, arouts, x_ts, L, sacc, qacc, wacc=None,
                            halves=range(2)):
                """Per AR half: DMA the landed data, add the residual
                in place (DVE), accumulate LN stats tile-wise on DVE:
                sacc += x, sq = x^2 (Scalar), qacc += sq, and optionally
                wacc += ws (.) x (Scalar per-partition scale + DVE)."""
                for hh in halves:
                    for s in range(4):
                        b = acts.tile([128, 4 * L], F16,
                                      name=f"arb_{tag}_{hh}_{s}", tag="arb",
                                      bufs=3, padded_shape=[128, 4 * 512])
                        nc.sync.dma_start(
                            b[:], arouts[hh][:, 4 * s * L:4 * (s + 1) * L])
                        g = 2 * hh + s // 2
                        off = (s % 2) * 4 * L
                        nc.vector.tensor_tensor(
                            x_ts[g][:, off:off + 4 * L], b[:],
                            x_ts[g][:, off:off + 4 * L], ALU.add)
                        for kk in range(4):
                            k = 16 * hh + 4 * s + kk
                            xk = xs(x_ts, L, k)
                            nc.vector.tensor_tensor(sacc[:], sacc[:], xk,
                                                    ALU.add)
                            sq = acts.tile([128, L], F16,
                                           name=f"sq_{tag}_{k}", tag="sqt",
                                           bufs=3, padded_shape=[128, 512])
                            nc.scalar.square(sq[:], xk)
                            nc.vector.tensor_tensor(qacc[:], qacc[:], sq[:],
                                                    ALU.add)
                            if wacc is not None:
                                wm = acts.tile([128, L], F16,
                                               name=f"wm_{tag}_{k}",
                                               tag="wmt", bufs=3,
                                               padded_shape=[128, 512])
                                nc.scalar.activation(wm[:], xk, AF.Copy,
                                                     scale=ws32[:, k:k + 1])
                                nc.vector.tensor_tensor(wacc[:], wacc[:],
                                                        wm[:], ALU.add)

            def ln_finalize(tag, s1p, s2p, L):
                mean = acts.tile([1, L], F32, name=f"mean_{tag}", tag="lmean")
                var = acts.tile([1, L], F32, name=f"var_{tag}", tag="lvar")
                tmpa = acts.tile([1, L], F32, name=f"tmpa_{tag}", tag="ltmp")
                r0 = acts.tile([1, L], F32, name=f"r0_{tag}", tag="lr0")
                nc.scalar.mul(mean[:], s1p, 1.0 / E)
                nc.scalar.mul(var[:], s2p, 1.0 / E)
                nc.scalar.square(tmpa[:], mean[:])
                nc.vector.tensor_sub(var[:], var[:], tmpa[:])
                nc.vector.tensor_scalar_add(var[:], var[:], 1e-5)
                nc.scalar.sqrt(tmpa[:], var[:])
                nc.vector.reciprocal(r0[:], tmpa[:])
                nc.vector.tensor_tensor(tmpa[:], r0[:], r0[:], ALU.mult)
                nc.vector.tensor_tensor(tmpa[:], tmpa[:], var[:], ALU.mult)
                nc.vector.tensor_scalar(tmpa[:], tmpa[:], -0.5, 1.5, ALU.mult,
                                        ALU.add)
                rstd = acts.tile([1, L], F16, name=f"rstd_{tag}", tag="rstd")
                nmr = acts.tile([1, L], F16, name=f"nmr_{tag}", tag="nmr")
                nc.vector.tensor_tensor(rstd[:], r0[:], tmpa[:], ALU.mult)
                nc.vector.scalar_tensor_tensor(nmr[:], mean[:], -1.0, rstd[:],
                                               ALU.mult, ALU.mult)
                Apsum = pb(f"ps_A_{tag}", L)
                nc.tensor.matmul(Apsum, ones_row[:], rstd[:], start=True,
                                 stop=True)
                Bpsum = pb(f"ps_B_{tag}", L)
                nc.tensor.matmul(Bpsum, ones_row[:], nmr[:], start=True,
                                 stop=True)
                Asb = acts.tile([128, L], F16, name=f"A_{tag}", tag="Asb")
                nc.scalar.copy(Asb[:], Apsum)
                Bsb = acts.tile([128, L], F16, name=f"B_{tag}", tag="Bsb")
                nc.scalar.copy(Bsb[:], Bpsum)
                return rstd, nmr, Asb, Bsb

            def stats_finalize(tag, sacc, qacc, L):
                q16 = acts.tile([128, L], F16, name=f"q16_{tag}",
                                tag=f"q16_{tag}")
                nc.scalar.copy(q16[:], qacc[:])
                s1p = pstat(f"ps_s1_{tag}", L)
                nc.tensor.matmul(s1p, ones_col[:], sacc[:], start=True,
                                 stop=True)
                s2p = pstat(f"ps_s2_{tag}", L)
                nc.tensor.matmul(s2p, ones_col[:], q16[:], start=True,
                                 stop=True)
                return ln_finalize(tag, s1p, s2p, L)

            def ln_apply(x_ts, L, Asb, Bsb, eng, dump=None):
                for k in range(ET):
                    eng.tensor_tensor(xs(x_ts, L, k), xs(x_ts, L, k),
                                      Asb[:], ALU.mult)
                    eng.tensor_tensor(xs(x_ts, L, k), xs(x_ts, L, k),
                                      Bsb[:], ALU.add)
                if dump is not None:
                    for g in range(ET // KG):
                        nc.sync.dma_start(
                            dump.ap()[:, KG * L * g:KG * L * (g + 1)],
                            x_ts[g][:])

            # ================= program =================
            cat_ts = load_x("catx", catp_d, ncat, ET // KG)

            # ---- all qkv projections first: AR1 then fires into a DMA-
            # quiet window (no weight-stream contention) and the CC queue
            # pipelines AR1 -> AR2 tightly ----
            q1 = proj_fm("q1", wd["q1"], cat_ts, ncat, "q1")
            rem_ts = load_x("remx", remp_d, nrem, ET // KG)
            k1 = proj_fm("k1", wd["k1"], cat_ts, ncat, "k1")
            v1 = proj_tm("v1", wd["v1"], cat_ts, ncat, JC, vtag="v1")
            q2 = proj_fm("q2", wd["q2"], rem_ts, nrem, "q")

            # ---- a1/op1 -> AR1 fires while k2/v2 still project ----
            o1 = attention("a1", q1, k1, v1, ncat, ncat, JC)
            arin1, arout1 = mk_ar("ar1", ncat)
            out_proj_to_dram("op1", o1, wd["o1"], arin1, arout1, ncat)

            k2 = proj_fm("k2", wd["k2"], rem_ts, nrem, "k")
            v2 = proj_tm("v2", wd["v2"], rem_ts, nrem, JR)
            o2 = attention("a2", q2, k2, v2, nrem, nrem, JR)
            arin2, arout2 = mk_ar("ar2", nrem)
            out_proj_to_dram("op2", o2, wd["o2"], arin2, arout2, nrem)

            # ---- x1 residual + stats chase (after a2's DVE softmax so
            # the DVE queue never blocks on AR1) ----
            sacc1, qacc1, _ = stat_acc("x1", ncat)
            resid_chase("x1", arout1, cat_ts, ncat, sacc1, qacc1)

            # ---- x1 LN finalize + apply (DVE), then kc/vc ----
            rstd1, nmr1, A1sb, B1sb = stats_finalize("x1", sacc1, qacc1,
                                                     ncat)
            ln_apply(cat_ts, ncat, A1sb, B1sb, nc.vector,
                     dump=dbg.get("dbg_x1"))
            kc = proj_fm("kc", wd["kc"], cat_ts, ncat, "k")
            vc = proj_tm("vc", wd["vc"], cat_ts, ncat, JC)

            # ---- r residual-add + stats chase (DVE adds feed qc_u) ----
            saccr, qaccr, _ = stat_acc("r", nrem)
            resid_chase("r", arout2, rem_ts, nrem, saccr, qaccr)

            # ---- qc projects RAW rsum chasing AR2; LN commuted ----
            rstdr_f = [None]

            qc_ps = []
            chunks_qc = []
            for g in range(ET // KG):
                wt = wchunk(f"w_qc_{g}", KG * DLOC)
                nc.sync.dma_start(
                    wt[:],
                    wd["qc"].ap()[:, KG * DLOC * g:KG * DLOC * (g + 1)])
                chunks_qc.append(wt)
            qc_ps = [pp(f"ps_qc_{m}", nrem) for m in range(4)]
            for k in range(ET):
                g, kk = k // KG, k % KG
                for m in range(4):
                    nc.tensor.matmul(
                        qc_ps[m],
                        chunks_qc[g][:, kk * DLOC + 128 * m:
                                     kk * DLOC + 128 * (m + 1)],
                        xs(rem_ts, nrem, k),
                        start=(k == 0), stop=(k == ET - 1))
            # r LN stats finalize (PE matmuls sit right after qc_u chase)
            rstdr, nmrr, Arsb, Brsb = stats_finalize("r", saccr, qaccr,
                                                     nrem)
            qc = []
            for m in range(4):
                r1p = pp(f"ps_r1_qc_{m}", nrem)
                nc.tensor.matmul(r1p, wq1r_sb[:, 128 * m:128 * (m + 1)],
                                 nmrr[:], start=True, stop=True)
                o = acts.tile([128, nrem], F16, name=f"qc_{m}", tag=f"q_{m}")
                nc.vector.tensor_tensor(o[:], qc_ps[m], Arsb[:], ALU.mult)
                nc.vector.tensor_tensor(o[:], o[:], r1p, ALU.add)
                qc.append(o)

            # ---- MHAc (q from r-normed, kv from x1) ----
            oc = attention("ac", qc, kc, vc, nrem, ncat, JC)
            arinc, aroutc = mk_ar("arc", nrem)
            out_proj_to_dram("opc", oc, wd["oc"], arinc, aroutc, nrem)

            # ---- materialize r-normed in place (x2 residual base);
            # Pool engine so the DVE stays free for the x2 chase ----
            ln_apply(rem_ts, nrem, Arsb, Brsb, nc.vector,
                     dump=dbg.get("dbg_r"))

            # ---- x2 = LN(r + ARc) chase; f1 chases quarter-wise m0..5,
            # LN commuted into the f1 pre-activations ----
            sacc2, qacc2, wacc2 = stat_acc("x2", nrem, want_w=True)

            chunks_f1 = []
            for g in range(ET // 4):
                wt = wchunk(f"w_f1_{g}", 4 * 768)
                nc.sync.dma_start(
                    wt[:],
                    wd["f1a"].ap()[:, 4 * 768 * g:4 * 768 * (g + 1)])
                chunks_f1.append(wt)

            ps6 = [pp(f"ps_f1_{m}", nrem) for m in range(6)]
            for hh in range(2):
                resid_chase("x2", aroutc, rem_ts, nrem, sacc2, qacc2,
                            wacc=wacc2, halves=[hh])
                for kk in range(16):
                    k = 16 * hh + kk
                    for m in range(6):
                        nc.tensor.matmul(
                            ps6[m],
                            chunks_f1[k // 4][:, (k % 4) * 768 + 128 * m:
                                              (k % 4) * 768 + 128 * (m + 1)],
                            xs(rem_ts, nrem, k),
                            start=(k == 0), stop=(k == ET - 1))
            rstd2, nmr2, A2sb, B2sb = stats_finalize("x2", sacc2, qacc2,
                                                     nrem)

            def f1_correct(m, psrc):
                u = acts.tile([128, nrem], F16, name=f"f1u_{m}", tag="f1u",
                              bufs=2, padded_shape=[128, 512])
                nc.vector.tensor_tensor(u[:], psrc, A2sb[:], ALU.mult)
                opp = pp(f"ps_f1o_{m}", nrem)
                nc.tensor.matmul(opp, f1rs_sb[:, 128 * m:128 * (m + 1)],
                                 nmr2[:], start=True, stop=True)
                nc.vector.tensor_tensor(u[:], u[:], opp, ALU.add)
                h = acts.tile([128, nrem], F16, name=f"hT_{m}", tag=f"hT_{m}")
                nc.scalar.activation(h[:], u[:], AF.Gelu)
                return h

            hT = [None] * 8
            for m in range(6):
                hT[m] = f1_correct(m, ps6[m])
            # m6/m7: all data resident, run back-to-back
            chunks_f1b = []
            for g in range(2):
                wt = wchunk(f"w_f1b_{g}", 16 * 256)
                nc.sync.dma_start(
                    wt[:], wd["f1b"].ap()[:, 4096 * g:4096 * (g + 1)])
                chunks_f1b.append(wt)
            ps2 = [pp(f"ps_f1b_{m}", nrem) for m in range(2)]
            for k in range(ET):
                for i in range(2):
                    nc.tensor.matmul(
                        ps2[i],
                        chunks_f1b[k // 16][:, (k % 16) * 256 + 128 * i:
                                            (k % 16) * 256 + 128 * (i + 1)],
                        xs(rem_ts, nrem, k),
                        start=(k == 0), stop=(k == ET - 1))
            for i, m in enumerate((6, 7)):
                hT[m] = f1_correct(m, ps2[i])

            # ---- wx2 = Ws . x2sum from the Pool accumulator ----
            w16 = acts.tile([128, nrem], F16, name="w16", tag="w16")
            nc.scalar.copy(w16[:], wacc2[:])
            wxu = pstat("ps_wx2", nrem)
            nc.tensor.matmul(wxu, ones_col[:], w16[:], start=True, stop=True)
            wx2 = acts.tile([1, nrem], F32, name="wx2", tag="wx2")
            nc.vector.tensor_tensor(wx2[:], wxu, rstd2[:], ALU.mult)
            nc.vector.scalar_tensor_tensor(wx2[:], nmr2[:],
                                           consts[0:1, 0:1], wx2[:],
                                           ALU.mult, ALU.add)

            # ---- linear logit stats from hT: s1 = c2.g ; ws += w2s.g/256 --
            c2p = pstat("ps_c2", nrem)
            for m in range(8):
                nc.tensor.matmul(c2p, c2w_sb[:, m:m + 1], hT[m][:],
                                 start=(m == 0), stop=(m == 7))
            w2p = pstat("ps_w2s", nrem)
            for m in range(8):
                nc.tensor.matmul(w2p, c2w_sb[:, 8 + m:9 + m], hT[m][:],
                                 start=(m == 0), stop=(m == 7))
            s1part = acts.tile([1, 512], F32, name="s1part", tag="s1part")
            wspart = acts.tile([1, 512], F32, name="wspart", tag="wspart")
            nc.vector.memset(s1part[:], 1.0)
            nc.vector.memset(wspart[:], 0.0)
            nc.vector.tensor_copy(s1part[:, 0:nrem], c2p)
            nc.vector.tensor_scalar(wx2[:], wx2[:], 1.0 / NCORES, 0.0,
                                    ALU.mult, ALU.add)
            nc.vector.scalar_tensor_tensor(wspart[:, 0:nrem], w2p,
                                           1.0 / 256.0, wx2[:],
                                           ALU.mult, ALU.add)
            arin6 = dram.tile([128, 16], F32, name="arin6", tag="arin6")
            arout6 = dram.tile([128, 16], F32, name="arout6", tag="arout6",
                               addr_space="Shared")
            nc.sync.dma_start(arin6[:, 0:4], s1part[:])
            nc.sync.dma_start(arin6[:, 4:8], wspart[:])
            nc.sync.dma_start(arin6[:, 12:16], s1part[:])

            # ---- materialize x2 in place for the f2 residual fold ----
            ln_apply(rem_ts, nrem, A2sb, B2sb, nc.vector,
                     dump=dbg.get("dbg_x2"))

            # ---- FFN f2: partial = x2/8 + Wf2_shard^T hT; RS/quarter ----
            arin4 = [dram.tile([128, 16 * nrem], F16, name=f"ar4i{q}",
                               tag=f"ar4i{q}") for q in range(2)]
            rs4 = [dram.tile([16, 16 * nrem], F16, name=f"rs4{q}",
                             tag=f"rs4{q}") for q in range(2)]
            for q in range(4):
                for sub in range(2):
                    wt = wchunk(f"w_f2_{q}_{sub}", 8 * 512)
                    nc.sync.dma_start(
                        wt[:], wd["f2"].ap()[:, 4096 * (2 * q + sub):
                                             4096 * (2 * q + sub + 1)])
                    ps = [pp(f"ps_f2_{q}_{sub}_{mi}", nrem)
                          for mi in range(4)]
                    for k in range(8):
                        for mi in range(4):
                            nc.tensor.matmul(
                                ps[mi],
                                wt[:, 512 * k + 128 * mi:
                                   512 * k + 128 * (mi + 1)],
                                hT[k][:],
                                start=(k == 0), stop=(k == 7))
                    st = acts.tile([128, 4 * nrem], F16,
                                   name=f"st_f2_{q}_{sub}",
                                   tag="stage", bufs=3,
                                   padded_shape=[128, 4 * 512])
                    for mi in range(4):
                        mm = 4 * sub + mi
                        m = 8 * q + mm
                        nc.vector.scalar_tensor_tensor(
                            st[:, mi * nrem:(mi + 1) * nrem],
                            xs(rem_ts, nrem, m), 1.0 / NCORES, ps[mi],
                            ALU.mult, ALU.add)
                    off = ((8 * (q % 2)) + 4 * sub) * nrem
                    nc.sync.dma_start(
                        arin4[q // 2][:, off:off + 4 * nrem], st[:])
                if q % 2 == 1:
                    nc.gpsimd.collective_compute(
                        "ReduceScatter", ALU.add, replica_groups=replica,
                        ins=[arin4[q // 2].opt()],
                        outs=[rs4[q // 2].opt()])

            # ---- s2 from the scattered summed features, chased/half ----
            CW = 4 * nrem  # columns per rs4 read chunk (4 chunks/half)
            s2p = pstat("ps_rs2", nrem)
            for cch in range(8):
                q, hcol = cch // 4, (cch % 4) * CW
                bt = acts.tile([16, CW], F16, name=f"rsb_{cch}", tag="rsb",
                               bufs=2, padded_shape=[16, 4 * 512])
                nc.sync.dma_start(bt[:], rs4[q][:, hcol:hcol + CW])
                sq = acts.tile([16, CW], F16, name=f"rssq_{cch}", tag="rssq",
                               bufs=2, padded_shape=[16, 4 * 512])
                nc.scalar.square(sq[:], bt[:])
                for s in range(4):
                    k = cch * 4 + s
                    nc.tensor.matmul(s2p, ones_col[0:16, :],
                                     sq[:, s * nrem:(s + 1) * nrem],
                                     start=(k == 0), stop=(k == ET - 1))
            s2part = acts.tile([1, 512], F32, name="s2part", tag="s2part")
            nc.vector.memset(s2part[:], 1.0)
            nc.vector.tensor_copy(s2part[:, 0:nrem], s2p)
            nc.sync.dma_start(arin6[:, 8:12], s2part[:])
            nc.gpsimd.collective_compute(
                "AllReduce", ALU.add, replica_groups=replica,
                ins=[arin6.opt()], outs=[arout6.opt()])

            # ---- final logit: affine-LN identity, 128-wide blocked ----
            gsb = acts.tile([128, 12], F32, name="gsb", tag="gsb")
            nc.sync.dma_start(gsb[:], arout6[:, 0:12])
            g1, g2, g3 = gsb[:, 0:4], gsb[:, 4:8], gsb[:, 8:12]
            mean = acts.tile([128, 4], F32, name="mean_l", tag="lmean_l")
            var = acts.tile([128, 4], F32, name="var_l", tag="lvar_l")
            tmpa = acts.tile([128, 4], F32, name="tmpa_l", tag="ltmp_l")
            r0 = acts.tile([128, 4], F32, name="r0_l", tag="lr0_l")
            nc.scalar.mul(mean[:], g1, 1.0 / E)
            nc.scalar.mul(var[:], g3, 1.0 / E)
            nc.scalar.square(tmpa[:], mean[:])
            nc.vector.tensor_sub(var[:], var[:], tmpa[:])
            nc.vector.tensor_scalar_add(var[:], var[:], 1e-5)
            nc.scalar.sqrt(tmpa[:], var[:])
            nc.vector.reciprocal(r0[:], tmpa[:])
            nc.vector.tensor_tensor(tmpa[:], r0[:], r0[:], ALU.mult)
            nc.vector.tensor_tensor(tmpa[:], tmpa[:], var[:], ALU.mult)
            nc.vector.tensor_scalar(tmpa[:], tmpa[:], -0.5, 1.5,
                                    ALU.mult, ALU.add)
            rstd = acts.tile([128, 4], F32, name="rstd_l", tag="rstd_l")
            nc.vector.tensor_tensor(rstd[:], r0[:], tmpa[:], ALU.mult)
            nmr = acts.tile([128, 4], F32, name="nmr_l", tag="nmr_l")
            nc.vector.scalar_tensor_tensor(nmr[:], mean[:], -1.0,
                                           rstd[:], ALU.mult, ALU.mult)
            wdot = acts.tile([128, 4], F32, name="wdot", tag="wdot")
            nc.vector.tensor_tensor(wdot[:], rstd[:], g2, ALU.mult)
            lsb = acts.tile([128, 4], F32, name="lsb", tag="lsb")
            nc.vector.scalar_tensor_tensor(lsb[:], nmr[:],
                                           consts[:, 0:1], wdot[:],
                                           ALU.mult, ALU.add)
            nc.sync.dma_start(logits_d.ap(), lsb[:])

    nc.compile()
    return nc


# ----------------------------------------------------------------------------
# host orchestration
# ----------------------------------------------------------------------------

def _packx(XT):
    """[E, L] fp32 -> [128, ET*L] fp16 feature-block pack."""
    L = XT.shape[1]
    return np.ascontiguousarray(
        XT.reshape(ET, 128, L).transpose(1, 0, 2).reshape(128, ET * L)
        .astype(np.float16))


def _prep_in_maps(vision_feature, text_embed, sel_idx, rem_idx,
                  Wqkv1, Wo1, Wqkv2, Wo2, Wqkvc, Woc, Wf1, Wf2, Ws):
    f16 = np.float16
    sel = vision_feature[sel_idx]
    rem = vision_feature[rem_idx]
    cat = np.concatenate([sel, text_embed], axis=0)

    remp = _packx(np.ascontiguousarray(rem.T))
    catp = _packx(np.ascontiguousarray(cat.T))
    ws_pack = np.ascontiguousarray(Ws[0].reshape(ET, 128).T.astype(f16))
    consts = np.broadcast_to(
        np.array([[np.float64(Ws.astype(np.float64).sum()), 0.0]],
                 np.float32), (128, 2)).copy()

    in_maps = []
    for c in range(NCORES):
        hs = slice(DLOC * c, DLOC * (c + 1))
        fs = slice(FLOC * c, FLOC * (c + 1))
        m = {"remp": remp, "catp": catp, "wsp": ws_pack, "consts": consts}
        for l, Wqkv, Wo in (("1", Wqkv1, Wo1), ("2", Wqkv2, Wo2),
                            ("c", Wqkvc, Woc)):
            Wq, Wk, Wv = Wqkv[:E], Wqkv[E:2 * E], Wqkv[2 * E:]
            for nm, W in (("q", Wq), ("k", Wk), ("v", Wv)):
                A = W[hs].T  # [E, DLOC]
                m[f"w{nm}{l}"] = np.ascontiguousarray(
                    A.reshape(ET, 128, DLOC).transpose(1, 0, 2)
                    .reshape(128, ET * DLOC).astype(f16))
            WoT = Wo[:, hs].T  # [DLOC, E]
            m[f"wo{l}"] = np.ascontiguousarray(
                WoT.reshape(4, 128, 4, 1024).transpose(1, 2, 0, 3)
                .reshape(128, 4 * E).astype(f16))
        # (Wq_c^T 1) for the commuted-LN fixup of qc
        m["wq1r"] = np.ascontiguousarray(
            Wqkvc[:E][hs].astype(np.float64).sum(axis=1).reshape(1, DLOC)
            .astype(f16))
        A = Wf1[fs].T  # [E, FLOC]
        m["wf1a"] = np.ascontiguousarray(
            A[:, 0:768].reshape(ET, 128, 768).transpose(1, 0, 2)
            .reshape(128, ET * 768).astype(f16))
        m["wf1b"] = np.ascontiguousarray(
            A[:, 768:1024].reshape(ET, 128, 256).transpose(1, 0, 2)
            .reshape(128, ET * 256).astype(f16))
        W2T = Wf2[:, fs].T  # [FLOC, E]
        m["wf2"] = np.ascontiguousarray(
            W2T.reshape(8, 128, 8, 512).transpose(1, 2, 0, 3)
            .reshape(128, 8 * E).astype(f16))
        c2 = Wf2[:, fs].astype(np.float64).sum(axis=0)  # [FLOC]
        w2s = 256.0 * (Ws[0].astype(np.float64) @ Wf2[:, fs].astype(np.float64))
        c2w = np.concatenate([c2.reshape(8, 128).T, w2s.reshape(8, 128).T],
                             axis=1)  # [128, 16]
        m["c2w"] = np.ascontiguousarray(c2w.astype(f16))
        m["f1rs"] = np.ascontiguousarray(
            Wf1[fs].astype(np.float64).sum(axis=1).reshape(1, FLOC)
            .astype(f16))
        in_maps.append(m)
    return in_maps


def run_device(in_maps, ncat_real, nrem_real, dumps=False, trace=False):
    from concourse.bass_utils import run_bass_kernel_spmd

    key = (ncat_real, nrem_real, dumps)
    if key not in _CACHE:
        _CACHE[key] = _build_device(ncat_real, nrem_real, dumps=dumps)
    nc = _CACHE[key]
    return run_bass_kernel_spmd(nc, in_maps, list(range(NCORES)), trace=trace)


def _kernel_impl(inputs, debug=False, trace=False):
    vision_feature = np.asarray(inputs["vision_feature"], np.float32)
    text_embed = np.asarray(inputs["text_embed"], np.float32)
    attention_mask = np.asarray(inputs["attention_mask"])

    biases_zero = all(
        not np.any(np.asarray(inputs[b]))
        for b in ("bqkv1", "bo1", "bqkv2", "bo2", "bqkvc", "boc",
                  "bf1", "bf2", "bs"))
    if (not bool(attention_mask.all())) or (not biases_zero):
        return (_reference_np(**{k: np.asarray(v) for k, v in inputs.items()}),
                None)

    t, sel_idx, rem_idx = _score_partition(vision_feature, text_embed,
                                           attention_mask)
    ncat_real = t + text_embed.shape[0]
    nrem_real = vision_feature.shape[0] - t
    kk = int(t * EXPAND)

    in_maps = _prep_in_maps(
        vision_feature, text_embed, sel_idx, rem_idx,
        np.asarray(inputs["Wqkv1"], np.float32),
        np.asarray(inputs["Wo1"], np.float32),
        np.asarray(inputs["Wqkv2"], np.float32),
        np.asarray(inputs["Wo2"], np.float32),
        np.asarray(inputs["Wqkvc"], np.float32),
        np.asarray(inputs["Woc"], np.float32),
        np.asarray(inputs["Wf1"], np.float32),
        np.asarray(inputs["Wf2"], np.float32),
        np.asarray(inputs["Ws"], np.float32))
    res = run_device(in_maps, ncat_real, nrem_real, dumps=debug, trace=trace)
    logits = res.results[0]["logits"][0, :nrem_real]
    es = (1.0 / (1.0 + np.exp(-logits.astype(np.float32))))
    ei = np.argsort(-es, kind="stable")[:kk]
    final = np.sort(np.concatenate([sel_idx, rem_idx[ei]]))
    return vision_feature[final], res


def kernel(**inputs):
    out, _ = _kernel_impl(inputs)
    return out


# revision 4
# speedup vs baseline: 1.0466x; 1.0203x over previous
"""Trainium2 Bass kernel for nn_CosSimRouter_learn_49778670960796. v2.

Schedule-restructured vs v1:
  * All big collectives quartered (one AR per out-proj feature quarter)
    and fired as soon as each quarter's staging lands; every consumer
    chases quarter-wise.
  * Program order: MHA1(cat) first (AR1 rides the empty early CC
    window), then MHA2(rem), then kc/vc, then qc which projects the RAW
    rem residual (r pre-LN) chasing AR2 quarters; the per-token LN
    scale/shift is commuted through the linear projection and applied
    as qc = rstd (x) qc_u + (Wq^T 1) (x) nmr afterwards (rank-1 via
    matmul + DVE).
  * LN statistics are accumulated on DVE/Pool/Scalar tile-wise while
    quarters land (no 32x ones-matmul chains on the PE): sacc (Pool,
    fp16), sq (Scalar) + qacc (DVE, fp32), then a single ones-matmul
    partition-reduce each.  Ws.x2sum (wx2) likewise via Scalar
    per-partition-scale copy + Pool fp32 accumulate.
  * f1 chases ARc quarters 6 psum banks wide (m0-5), m6/m7 run
    back-to-back after the last quarter; PSUM ring split 6 ("pp") + 2
    ("pps" for [1,L] stats / LN broadcast psums).
  * f2 fires one ReduceScatter per output-feature quarter; the s2
    stat chase + tiny fp32 AR (s1/ws/s2) close the kernel.
"""

import numpy as np

E = 4096
H = 16
HID = 8192
GAMMA = 0.2
TEMP = 0.05
EXPAND = 0.7
NCORES = 8
ET = E // 128  # 32 feature tiles
DH = E // H  # 256
HL = H // NCORES  # 2 heads per core
DLOC = HL * DH  # 512 local head dims
FLOC = HID // NCORES  # 1024 local ffn hidden
KG = 8  # k-blocks per weight/act chunk (== one AR quarter)

_CACHE = {}


# ----------------------------------------------------------------------------
# host-side reference math (numpy, fp32) for the scoring stage + fallback
# ----------------------------------------------------------------------------

def _score_partition(vision_feature, text_embed, attention_mask):
    vf = vision_feature.astype(np.float32)
    te = text_embed.astype(np.float32)
    vn = vf / np.maximum(np.linalg.norm(vf, axis=-1, keepdims=True), 1e-8)
    tn = te / np.maximum(np.linalg.norm(te, axis=-1, keepdims=True), 1e-8)
    cs = vn @ tn.T
    cs = np.where(attention_mask[None, :], cs, np.float32(0.0))
    m = cs.max(axis=-1) / np.float32(TEMP)
    e = np.exp(m - m.max())
    scores = e / e.sum()
    order = np.argsort(-scores, kind="stable")
    cum = np.cumsum(scores[order])
    t = int((cum <= GAMMA).sum())
    return t, order[:t], order[t:]


def _ln_np(x):
    m = x.mean(-1, keepdims=True)
    v = ((x - m) ** 2).mean(-1, keepdims=True)
    return (x - m) / np.sqrt(v + 1e-5)


def _gelu_np(x):
    import math

    erf = np.frompyfunc(math.erf, 1, 1)
    return (x * 0.5 * (1.0 + erf(x / math.sqrt(2.0)).astype(np.float64))
            ).astype(x.dtype)


def _mha_np(q_in, kv_in, Wqkv, bqkv, Wo, bo):
    dh = E // H
    Wq, Wk, Wv = np.split(Wqkv, 3, axis=0)
    bq, bk, bv = np.split(bqkv, 3)
    q = (q_in @ Wq.T + bq).reshape(-1, H, dh)
    k = (kv_in @ Wk.T + bk).reshape(-1, H, dh)
    v = (kv_in @ Wv.T + bv).reshape(-1, H, dh)
    att = np.einsum("qhd,khd->hqk", q, k) / np.float32(np.sqrt(dh))
    att = att - att.max(-1, keepdims=True)
    att = np.exp(att)
    att /= att.sum(-1, keepdims=True)
    o = np.einsum("hqk,khd->qhd", att.astype(np.float32), v).reshape(-1, E)
    return o @ Wo.T + bo


def _reference_np(vision_feature, text_embed, attention_mask,
                  Wqkv1, bqkv1, Wo1, bo1, Wqkv2, bqkv2, Wo2, bo2,
                  Wqkvc, bqkvc, Woc, boc, Wf1, bf1, Wf2, bf2, Ws, bs):
    t, sel_idx, rem_idx = _score_partition(vision_feature, text_embed,
                                           attention_mask)
    sel = vision_feature[sel_idx]
    rem = vision_feature[rem_idx]
    cat = np.concatenate([sel, text_embed], axis=0)
    x = _ln_np(_mha_np(cat, cat, Wqkv1, bqkv1, Wo1, bo1) + cat)
    r = _ln_np(_mha_np(rem, rem, Wqkv2, bqkv2, Wo2, bo2) + rem)
    x = _ln_np(_mha_np(r, x, Wqkvc, bqkvc, Woc, boc) + r)
    ffn = _gelu_np(x @ Wf1.T + bf1) @ Wf2.T + bf2
    x = _ln_np(x + ffn)
    logits = (x @ Ws.T + bs).squeeze(-1)
    es = 1.0 / (1.0 + np.exp(-logits))
    k = int(t * EXPAND)
    ei = np.argsort(-es, kind="stable")[:k]
    final = np.sort(np.concatenate([sel_idx, rem_idx[ei]]))
    return vision_feature[final]


# ----------------------------------------------------------------------------
# device program
# ----------------------------------------------------------------------------

def _build_device(ncat, nrem, dumps=False):
    import concourse.bacc as bacc
    import concourse.mybir as mybir
    import concourse.tile as tile

    dt = mybir.dt
    F32 = dt.float32
    F16 = dt.float16
    AF = mybir.ActivationFunctionType
    ALU = mybir.AluOpType

    JC = (ncat + 127) // 128  # kv partition tiles for cat (2)
    JR = (nrem + 127) // 128  # kv partition tiles for rem (4)

    nc = bacc.Bacc("TRN2", target_bir_lowering=False, debug=False,
                   num_devices=NCORES)

    # ---------------- DRAM I/O (all host-packed, see _prep_in_maps) --------
    remp_d = nc.dram_tensor("remp", [128, ET * nrem], F16, kind="ExternalInput")
    catp_d = nc.dram_tensor("catp", [128, ET * ncat], F16, kind="ExternalInput")
    wd = {}
    for l in ("1", "2", "c"):
        for p in ("q", "k", "v"):
            wd[p + l] = nc.dram_tensor(f"w{p}{l}", [128, ET * DLOC], F16,
                                       kind="ExternalInput")
        wd["o" + l] = nc.dram_tensor(f"wo{l}", [128, (DLOC // 128) * E], F16,
                                     kind="ExternalInput")
    wd["f1a"] = nc.dram_tensor("wf1a", [128, ET * 768], F16,
                               kind="ExternalInput")
    wd["f1b"] = nc.dram_tensor("wf1b", [128, ET * 256], F16,
                               kind="ExternalInput")
    wd["f2"] = nc.dram_tensor("wf2", [128, (FLOC // 128) * E], F16,
                              kind="ExternalInput")
    ws_d = nc.dram_tensor("wsp", [128, ET], F16, kind="ExternalInput")
    c2w_d = nc.dram_tensor("c2w", [128, 2 * (FLOC // 128)], F16,
                           kind="ExternalInput")
    f1rs_d = nc.dram_tensor("f1rs", [1, FLOC], F16, kind="ExternalInput")
    wq1r_d = nc.dram_tensor("wq1r", [1, DLOC], F16, kind="ExternalInput")
    consts_d = nc.dram_tensor("consts", [128, 2], F32, kind="ExternalInput")
    statso_d = nc.dram_tensor("statso", [1, 1536], F32,
                              kind="ExternalOutput")
    dbg = {}
    if dumps:
        for nm, L in (("dbg_x1", ncat), ("dbg_r", nrem), ("dbg_x2", nrem)):
            dbg[nm] = nc.dram_tensor(nm, [128, ET * L], F16,
                                     kind="ExternalOutput")

    replica = [list(range(NCORES))]

    with tile.TileContext(nc, num_cores=NCORES) as tc:
        with (
            tc.tile_pool(name="acts", bufs=1) as acts,
            tc.tile_pool(name="psum", bufs=1, space="PSUM") as psum,
            tc.tile_pool(name="dram", bufs=1, space="DRAM") as dram,
        ):
            # ---- constants ----
            ones_col = acts.tile([128, 1], F16, name="ones_col",
                                 tag="ones_col")
            nc.vector.memset(ones_col[:], 1.0)
            ones_row = acts.tile([1, 128], F16, name="ones_row",
                                 tag="ones_row")
            nc.vector.memset(ones_row[:], 1.0)
            ws_sb = acts.tile([128, ET], F16, name="ws_sb", tag="ws_sb")
            nc.sync.dma_start(ws_sb[:], ws_d.ap())
            ws32 = acts.tile([128, ET], F32, name="ws32", tag="ws32")
            nc.vector.tensor_copy(ws32[:], ws_sb[:])
            c2w_sb = acts.tile([128, 2 * (FLOC // 128)], F16, name="c2w_sb",
                               tag="c2w_sb")
            nc.sync.dma_start(c2w_sb[:], c2w_d.ap())
            consts = acts.tile([128, 2], F32, name="consts", tag="consts")
            nc.sync.dma_start(consts[:], consts_d.ap())
            f1rs_sb = acts.tile([1, FLOC], F16, name="f1rs_sb", tag="f1rs")
            nc.sync.dma_start(f1rs_sb[:], f1rs_d.ap())
            wq1r_sb = acts.tile([1, DLOC], F16, name="wq1r_sb", tag="wq1r")
            nc.sync.dma_start(wq1r_sb[:], wq1r_d.ap())

            # PSUM ring: 6 full banks ("pp") + 2 banks for [1,L] stats and
            # LN broadcast psums ("pps"). 6*2048 + 2*2048 = 16 KiB.
            def pp(name, L, parts=128):
                t_ = psum.tile([128, L], F32, name=name, tag="pp", bufs=6,
                               padded_shape=[128, 512])
                return t_[0:parts, :] if parts < 128 else t_[:]

            def pstat(name, L):
                return psum.tile([1, L], F32, name=name, tag="pps", bufs=1,
                                 padded_shape=[1, 512])[:]

            def pb(name, L):
                return psum.tile([128, L], F32, name=name, tag="ppb",
                                 bufs=1, padded_shape=[128, 512])[:]

            def wchunk(name, cols):
                return acts.tile([128, cols], F16, name=name, tag="wt",
                                 bufs=3, padded_shape=[128, 4096])

            # ---- activations: group tiles + slice helper ----
            def load_x(name, dram_t, L, ngroups):
                ts = []
                for g in range(ngroups):
                    xt = acts.tile([128, KG * L], F16, name=f"{name}_{g}",
                                   tag=f"{name}_{g}")
                    nc.sync.dma_start(
                        xt[:], dram_t.ap()[:, KG * L * g:KG * L * (g + 1)])
                    ts.append(xt)
                return ts

            def xs(ts, L, k):
                g, kk = k // KG, k % KG
                return ts[g][:, kk * L:(kk + 1) * L]

            # ---------------- building blocks ----------------
            def proj_fm(tagbase, w_dram, x_ts, L, outtag, correct=None):
                """q/k projection -> 4 tiles [128, L] fp16 (DLOC, L) layout.

                correct=(Asb, nmr_row, w1r_sb): instead of a plain PSUM
                copy, apply the commuted-LN fixup
                out_m = Asb (x) psum_m + (w1r_m (x) nmr).
                """
                chunks = []
                for g in range(ET // KG):
                    wt = wchunk(f"w_{tagbase}_{g}", KG * DLOC)
                    nc.sync.dma_start(
                        wt[:],
                        w_dram.ap()[:, KG * DLOC * g:KG * DLOC * (g + 1)])
                    chunks.append(wt)
                ps = [pp(f"ps_{tagbase}_{m}", L) for m in range(4)]
                for k in range(ET):
                    g, kk = k // KG, k % KG
                    for m in range(4):
                        nc.tensor.matmul(
                            ps[m],
                            chunks[g][:, kk * DLOC + 128 * m:
                                      kk * DLOC + 128 * (m + 1)],
                            xs(x_ts, L, k),
                            start=(k == 0), stop=(k == ET - 1))
                outs = []
                if correct is None:
                    for m in range(4):
                        o = acts.tile([128, L], F16, name=f"{tagbase}_{m}",
                                      tag=f"{outtag}_{m}")
                        nc.scalar.copy(o[:], ps[m])
                        outs.append(o)
                else:
                    Asb, nmr_row, w1r = correct
                    for m in range(4):
                        r1p = pp(f"ps_r1_{tagbase}_{m}", L)
                        nc.tensor.matmul(r1p,
                                         w1r[:, 128 * m:128 * (m + 1)],
                                         nmr_row[:], start=True, stop=True)
                        o = acts.tile([128, L], F16, name=f"{tagbase}_{m}",
                                      tag=f"{outtag}_{m}")
                        nc.vector.tensor_tensor(o[:], ps[m], Asb[:], ALU.mult)
                        nc.vector.tensor_tensor(o[:], o[:], r1p, ALU.add)
                        outs.append(o)
                return outs

            def proj_tm(tagbase, w_dram, x_ts, L, JT, vtag="v"):
                """v projection -> JT tiles [128, DLOC] fp16 (kv, DLOC)."""
                chunks = []
                for g in range(ET // KG):
                    wt = wchunk(f"w_{tagbase}_{g}", KG * DLOC)
                    nc.sync.dma_start(
                        wt[:],
                        w_dram.ap()[:, KG * DLOC * g:KG * DLOC * (g + 1)])
                    chunks.append(wt)
                ps = []
                for j in range(JT):
                    pj = min(128, L - 128 * j)
                    ps.append(pp(f"ps_{tagbase}_{j}", DLOC, parts=pj))
                for k in range(ET):
                    g, kk = k // KG, k % KG
                    for j in range(JT):
                        pj = min(128, L - 128 * j)
                        nc.tensor.matmul(
                            ps[j],
                            xs(x_ts, L, k)[:, 128 * j:128 * j + pj],
                            chunks[g][:, kk * DLOC:(kk + 1) * DLOC],
                            start=(k == 0), stop=(k == ET - 1))
                outs = []
                for j in range(JT):
                    pj = min(128, L - 128 * j)
                    o = acts.tile([128, DLOC], F16, name=f"{tagbase}_{j}",
                                  tag=f"{vtag}_{j}")
                    nc.scalar.copy(o[0:pj, :], ps[j])
                    outs.append(o)
                return outs

            def attention(tag, qT, kT, vT, Lq, Lkv, JT):
                exps_h = []
                for h in range(HL):
                    exps = []
                    for j in range(JT):
                        pj = min(128, Lkv - 128 * j)
                        p = pp(f"ps_s_{tag}_{h}_{j}", Lq, parts=pj)
                        for c in range(2):
                            nc.tensor.matmul(
                                p,
                                kT[2 * h + c][:, 128 * j:128 * j + pj],
                                qT[2 * h + c][:],
                                start=(c == 0), stop=(c == 1))
                        e = acts.tile([128, Lq], F16,
                                      name=f"es_{tag}_{h}_{j}",
                                      tag=f"expS_{h}_{j}")
                        nc.scalar.activation(e[0:pj, :], p, AF.Exp,
                                             scale=float(1.0 / np.sqrt(DH)))
                        exps.append(e)
                    exps_h.append(exps)
                rec2s = []
                for h in range(HL):
                    dsum = pstat(f"ps_d_{tag}_{h}", Lq)
                    for j in range(JT):
                        pj = min(128, Lkv - 128 * j)
                        nc.tensor.matmul(dsum, ones_col[0:pj, :],
                                         exps_h[h][j][0:pj, :],
                                         start=(j == 0), stop=(j == JT - 1))
                    den = acts.tile([1, Lq], F32, name=f"den_{tag}_{h}",
                                    tag="aden")
                    rec = acts.tile([1, Lq], F32, name=f"rec_{tag}_{h}",
                                    tag="arec")
                    nc.vector.tensor_copy(den[:], dsum)
                    nc.vector.reciprocal(rec[:], den[:])
                    nc.vector.tensor_tensor(den[:], den[:], rec[:], ALU.mult)
                    nc.vector.tensor_scalar(den[:], den[:], -1.0, 2.0,
                                            ALU.mult, ALU.add)
                    rec2 = acts.tile([1, Lq], F16, name=f"rec2_{tag}_{h}",
                                     tag=f"rec2_{h}")
                    nc.vector.tensor_tensor(rec2[:], rec[:], den[:], ALU.mult)
                    rec2s.append(rec2)
                pos = []
                for h in range(HL):
                    for c in range(2):
                        po = pp(f"ps_o_{tag}_{h}_{c}", Lq)
                        for j in range(JT):
                            pj = min(128, Lkv - 128 * j)
                            nc.tensor.matmul(
                                po,
                                vT[j][0:pj, 256 * h + 128 * c:
                                      256 * h + 128 * (c + 1)],
                                exps_h[h][j][0:pj, :],
                                start=(j == 0), stop=(j == JT - 1))
                        pos.append(po)
                oT = []
                for h in range(HL):
                    rrep_p = pp(f"ps_rr_{tag}_{h}", Lq)
                    nc.tensor.matmul(rrep_p, ones_row[:], rec2s[h][:],
                                     start=True, stop=True)
                    rrep = acts.tile([128, Lq], F32, name=f"rr_{tag}_{h}",
                                     tag=f"rrep_{h}")
                    nc.scalar.copy(rrep[:], rrep_p)
                    for c in range(2):
                        o = acts.tile([128, Lq], F16,
                                      name=f"oT_{tag}_{h}_{c}",
                                      tag=f"oT_{2 * h + c}")
                        nc.vector.tensor_tensor(o[:], pos[2 * h + c],
                                                rrep[:], ALU.mult)
                        oT.append(o)
                return oT

            def out_proj_to_dram(tag, oT, w_dram, arins, arouts, Lq):
                """Out-proj in feature quarters (4-wide psum groups,
                Scalar staging copies), one AllReduce per HALF."""
                for q in range(4):
                    wt = wchunk(f"wo_{tag}_{q}", 4 * 1024)
                    nc.sync.dma_start(
                        wt[:], w_dram.ap()[:, 4096 * q:4096 * (q + 1)])
                    for sub in range(2):
                        ps = [pp(f"ps_op_{tag}_{q}_{sub}_{mi}", Lq)
                              for mi in range(4)]
                        for k in range(4):
                            for mi in range(4):
                                mm = 4 * sub + mi
                                nc.tensor.matmul(
                                    ps[mi],
                                    wt[:, 1024 * k + 128 * mm:
                                       1024 * k + 128 * (mm + 1)],
                                    oT[k][:],
                                    start=(k == 0), stop=(k == 3))
                        st = acts.tile([128, 4 * Lq], F16,
                                       name=f"st_{tag}_{q}_{sub}",
                                       tag="stage", bufs=3,
                                       padded_shape=[128, 4 * 512])
                        for mi in range(4):
                            nc.scalar.copy(
                                st[:, mi * Lq:(mi + 1) * Lq], ps[mi])
                        off = ((8 * (q % 2)) + 4 * sub) * Lq
                        nc.sync.dma_start(
                            arins[q // 2][:, off:off + 4 * Lq], st[:])
                    if q % 2 == 1:
                        nc.gpsimd.collective_compute(
                            "AllReduce", ALU.add, replica_groups=replica,
                            ins=[arins[q // 2].opt()],
                            outs=[arouts[q // 2].opt()])

            def mk_ar(tag, L, n=2):
                arins = [dram.tile([128, 16 * L], F16, name=f"{tag}i{q}",
                                   tag=f"{tag}i{q}") for q in range(n)]
                arouts = [dram.tile([128, 16 * L], F16, name=f"{tag}o{q}",
                                    tag=f"{tag}o{q}", addr_space="Shared")
                          for q in range(n)]
                return arins, arouts

            def stat_acc(tag, L, want_w=False):
                sacc = acts.tile([128, L], F16, name=f"sacc_{tag}",
                                 tag=f"sacc_{tag}")
                nc.vector.memset(sacc[:], 0.0)
                qacc = acts.tile([128, L], F32, name=f"qacc_{tag}",
                                 tag=f"qacc_{tag}")
                nc.vector.memset(qacc[:], 0.0)
                wacc = None
                if want_w:
                    wacc = acts.tile([128, L], F32, name=f"wacc_{tag}",
                                     tag=f"wacc_{tag}")
                    nc.vector.memset(wacc[:], 0.0)
                return sacc, qacc, wacc

            def resid_chase(tag, arouts, x_ts, L, sacc, qacc, wacc=None,
                            halves=range(2)):
                """Per AR half: DMA the landed data, add the residual
                in place (DVE), accumulate LN stats tile-wise on DVE:
                sacc += x, sq = x^2 (Scalar), qacc += sq, and optionally
                wacc += ws (.) x (Scalar per-partition scale + DVE)."""
                for hh in halves:
                    for s in range(4):
                        b = acts.tile([128, 4 * L], F16,
                                      name=f"arb_{tag}_{hh}_{s}", tag="arb",
                                      bufs=3, padded_shape=[128, 4 * 512])
                        nc.sync.dma_start(
                            b[:], arouts[hh][:, 4 * s * L:4 * (s + 1) * L])
                        g = 2 * hh + s // 2
                        off = (s % 2) * 4 * L
                        nc.vector.tensor_tensor(
                            x_ts[g][:, off:off + 4 * L], b[:],
                            x_ts[g][:, off:off + 4 * L], ALU.add)
                        for kk in range(4):
                            k = 16 * hh + 4 * s + kk
                            xk = xs(x_ts, L, k)
                            nc.vector.tensor_tensor(sacc[:], sacc[:], xk,
                                                    ALU.add)
                            sq = acts.tile([128, L], F16,
                                           name=f"sq_{tag}_{k}", tag="sqt",
                                           bufs=3, padded_shape=[128, 512])
                            nc.scalar.square(sq[:], xk)
                            nc.vector.tensor_tensor(qacc[:], qacc[:], sq[:],
                                                    ALU.add)
                            if wacc is not None:
                                wm = acts.tile([128, L], F16,
                                               name=f"wm_{tag}_{k}",
                                               tag="wmt", bufs=3,
                                               padded_shape=[128, 512])
                                nc.scalar.activation(wm[:], xk, AF.Copy,
                                                     scale=ws32[:, k:k + 1])
                                nc.vector.tensor_tensor(wacc[:], wacc[:],
                                                        wm[:], ALU.add)

            def ln_finalize(tag, s1p, s2p, L):
                mean = acts.tile([1, L], F32, name=f"mean_{tag}", tag="lmean")
                var = acts.tile([1, L], F32, name=f"var_{tag}", tag="lvar")
                tmpa = acts.tile([1, L], F32, name=f"tmpa_{tag}", tag="ltmp")
                r0 = acts.tile([1, L], F32, name=f"r0_{tag}", tag="lr0")
                nc.scalar.mul(mean[:], s1p, 1.0 / E)
                nc.scalar.mul(var[:], s2p, 1.0 / E)
                nc.scalar.square(tmpa[:], mean[:])
                nc.vector.tensor_sub(var[:], var[:], tmpa[:])
                nc.vector.tensor_scalar_add(var[:], var[:], 1e-5)
                nc.scalar.sqrt(tmpa[:], var[:])
                nc.vector.reciprocal(r0[:], tmpa[:])
                nc.vector.tensor_tensor(tmpa[:], r0[:], r0[:], ALU.mult)
                nc.vector.tensor_tensor(tmpa[:], tmpa[:], var[:], ALU.mult)
                nc.vector.tensor_scalar(tmpa[:], tmpa[:], -0.5, 1.5, ALU.mult,
                                        ALU.add)
                rstd = acts.tile([1, L], F16, name=f"rstd_{tag}", tag="rstd")
                nmr = acts.tile([1, L], F16, name=f"nmr_{tag}", tag="nmr")
                nc.vector.tensor_tensor(rstd[:], r0[:], tmpa[:], ALU.mult)
                nc.vector.scalar_tensor_tensor(nmr[:], mean[:], -1.0, rstd[:],
                                               ALU.mult, ALU.mult)
                Apsum = pb(f"ps_A_{tag}", L)
                nc.tensor.matmul(Apsum, ones_row[:], rstd[:], start=True,
                                 stop=True)
                Bpsum = pb(f"ps_B_{tag}", L)
                nc.tensor.matmul(Bpsum, ones_row[:], nmr[:], start=True,
                                 stop=True)
                Asb = acts.tile([128, L], F16, name=f"A_{tag}", tag="Asb")
                nc.scalar.copy(Asb[:], Apsum)
                Bsb = acts.tile([128, L], F16, name=f"B_{tag}", tag="Bsb")
                nc.scalar.copy(Bsb[:], Bpsum)
                return rstd, nmr, Asb, Bsb

            def stats_finalize(tag, sacc, qacc, L):
                q16 = acts.tile([128, L], F16, name=f"q16_{tag}",
                                tag=f"q16_{tag}")
                nc.scalar.copy(q16[:], qacc[:])
                s1p = pstat(f"ps_s1_{tag}", L)
                nc.tensor.matmul(s1p, ones_col[:], sacc[:], start=True,
                                 stop=True)
                s2p = pstat(f"ps_s2_{tag}", L)
                nc.tensor.matmul(s2p, ones_col[:], q16[:], start=True,
                                 stop=True)
                return ln_finalize(tag, s1p, s2p, L)

            def ln_apply(x_ts, L, Asb, Bsb, eng, dump=None):
                for k in range(ET):
                    eng.tensor_tensor(xs(x_ts, L, k), xs(x_ts, L, k),
                                      Asb[:], ALU.mult)
                    eng.tensor_tensor(xs(x_ts, L, k), xs(x_ts, L, k),
                                      Bsb[:], ALU.add)
                if dump is not None:
                    for g in range(ET // KG):
                        nc.sync.dma_start(
                            dump.ap()[:, KG * L * g:KG * L * (g + 1)],
                            x_ts[g][:])

            # ================= program =================
            cat_ts = load_x("catx", catp_d, ncat, ET // KG)

            # ---- all qkv projections first: AR1 then fires into a DMA-
            # quiet window (no weight-stream contention) and the CC queue
            # pipelines AR1 -> AR2 tightly ----
            q1 = proj_fm("q1", wd["q1"], cat_ts, ncat, "q1")
            rem_ts = load_x("remx", remp_d, nrem, ET // KG)
            k1 = proj_fm("k1", wd["k1"], cat_ts, ncat, "k1")
            v1 = proj_tm("v1", wd["v1"], cat_ts, ncat, JC, vtag="v1")
            q2 = proj_fm("q2", wd["q2"], rem_ts, nrem, "q")

            # ---- a1/op1 -> AR1 fires while k2/v2 still project ----
            o1 = attention("a1", q1, k1, v1, ncat, ncat, JC)
            arin1, arout1 = mk_ar("ar1", ncat)
            out_proj_to_dram("op1", o1, wd["o1"], arin1, arout1, ncat)

            k2 = proj_fm("k2", wd["k2"], rem_ts, nrem, "k")
            v2 = proj_tm("v2", wd["v2"], rem_ts, nrem, JR)
            o2 = attention("a2", q2, k2, v2, nrem, nrem, JR)
            arin2, arout2 = mk_ar("ar2", nrem)
            out_proj_to_dram("op2", o2, wd["o2"], arin2, arout2, nrem)

            # ---- x1 residual + stats chase (after a2's DVE softmax so
            # the DVE queue never blocks on AR1) ----
            sacc1, qacc1, _ = stat_acc("x1", ncat)
            resid_chase("x1", arout1, cat_ts, ncat, sacc1, qacc1)

            # ---- x1 LN finalize + apply (DVE), then kc/vc ----
            rstd1, nmr1, A1sb, B1sb = stats_finalize("x1", sacc1, qacc1,
                                                     ncat)
            ln_apply(cat_ts, ncat, A1sb, B1sb, nc.vector,
                     dump=dbg.get("dbg_x1"))
            kc = proj_fm("kc", wd["kc"], cat_ts, ncat, "k")
            vc = proj_tm("vc", wd["vc"], cat_ts, ncat, JC)

            # ---- r residual-add + stats chase (DVE adds feed qc_u) ----
            saccr, qaccr, _ = stat_acc("r", nrem)
            resid_chase("r", arout2, rem_ts, nrem, saccr, qaccr)

            # ---- qc projects RAW rsum chasing AR2; LN commuted ----
            rstdr_f = [None]

            qc_ps = []
            chunks_qc = []
            for g in range(ET // KG):
                wt = wchunk(f"w_qc_{g}", KG * DLOC)
                nc.sync.dma_start(
                    wt[:],
                    wd["qc"].ap()[:, KG * DLOC * g:KG * DLOC * (g + 1)])
                chunks_qc.append(wt)
            qc_ps = [pp(f"ps_qc_{m}", nrem) for m in range(4)]
            for k in range(ET):
                g, kk = k // KG, k % KG
                for m in range(4):
                    nc.tensor.matmul(
                        qc_ps[m],
                        chunks_qc[g][:, kk * DLOC + 128 * m:
                                     kk * DLOC + 128 * (m + 1)],
                        xs(rem_ts, nrem, k),
                        start=(k == 0), stop=(k == ET - 1))
            # r LN stats finalize (PE matmuls sit right after qc_u chase)
            rstdr, nmrr, Arsb, Brsb = stats_finalize("r", saccr, qaccr,
                                                     nrem)
            qc = []
            for m in range(4):
                r1p = pp(f"ps_r1_qc_{m}", nrem)
                nc.tensor.matmul(r1p, wq1r_sb[:, 128 * m:128 * (m + 1)],
                                 nmrr[:], start=True, stop=True)
                o = acts.tile([128, nrem], F16, name=f"qc_{m}", tag=f"q_{m}")
                nc.vector.tensor_tensor(o[:], qc_ps[m], Arsb[:], ALU.mult)
                nc.vector.tensor_tensor(o[:], o[:], r1p, ALU.add)
                qc.append(o)

            # ---- MHAc (q from r-normed, kv from x1) ----
            oc = attention("ac", qc, kc, vc, nrem, ncat, JC)
            arinc, aroutc = mk_ar("arc", nrem)
            out_proj_to_dram("opc", oc, wd["oc"], arinc, aroutc, nrem)

            # ---- materialize r-normed in place (x2 residual base);
            # Pool engine so the DVE stays free for the x2 chase ----
            ln_apply(rem_ts, nrem, Arsb, Brsb, nc.vector,
                     dump=dbg.get("dbg_r"))

            # ---- x2 = LN(r + ARc) chase; f1 chases quarter-wise m0..5,
            # LN commuted into the f1 pre-activations ----
            sacc2, qacc2, wacc2 = stat_acc("x2", nrem, want_w=True)

            chunks_f1 = []
            for g in range(ET // 4):
                wt = wchunk(f"w_f1_{g}", 4 * 768)
                nc.sync.dma_start(
                    wt[:],
                    wd["f1a"].ap()[:, 4 * 768 * g:4 * 768 * (g + 1)])
                chunks_f1.append(wt)

            ps6 = [pp(f"ps_f1_{m}", nrem) for m in range(6)]
            for hh in range(2):
                resid_chase("x2", aroutc, rem_ts, nrem, sacc2, qacc2,
                            wacc=wacc2, halves=[hh])
                for kk in range(16):
                    k = 16 * hh + kk
                    for m in range(6):
                        nc.tensor.matmul(
                            ps6[m],
                            chunks_f1[k // 4][:, (k % 4) * 768 + 128 * m:
                                              (k % 4) * 768 + 128 * (m + 1)],
                            xs(rem_ts, nrem, k),
                            start=(k == 0), stop=(k == ET - 1))
            rstd2, nmr2, A2sb, B2sb = stats_finalize("x2", sacc2, qacc2,
                                                     nrem)

            def f1_correct(m, psrc):
                u = acts.tile([128, nrem], F16, name=f"f1u_{m}", tag="f1u",
                              bufs=2, padded_shape=[128, 512])
                nc.vector.tensor_tensor(u[:], psrc, A2sb[:], ALU.mult)
                opp = pp(f"ps_f1o_{m}", nrem)
                nc.tensor.matmul(opp, f1rs_sb[:, 128 * m:128 * (m + 1)],
                                 nmr2[:], start=True, stop=True)
                nc.vector.tensor_tensor(u[:], u[:], opp, ALU.add)
                h = acts.tile([128, nrem], F16, name=f"hT_{m}", tag=f"hT_{m}")
                nc.scalar.activation(h[:], u[:], AF.Gelu)
                return h

            hT = [None] * 8
            for m in range(6):
                hT[m] = f1_correct(m, ps6[m])
            # m6/m7: all data resident, run back-to-back
            chunks_f1b = []
            for g in range(2):
                wt = wchunk(f"w_f1b_{g}", 16 * 256)
                nc.sync.dma_start(
                    wt[:], wd["f1b"].ap()[:, 4096 * g:4096 * (g + 1)])
                chunks_f1b.append(wt)
            ps2 = [pp(f"ps_f1b_{m}", nrem) for m in range(2)]
            for k in range(ET):
                for i in range(2):
                    nc.tensor.matmul(
                        ps2[i],
                        chunks_f1b[k // 16][:, (k % 16) * 256 + 128 * i:
                                            (k % 16) * 256 + 128 * (i + 1)],
                        xs(rem_ts, nrem, k),
                        start=(k == 0), stop=(k == ET - 1))
            for i, m in enumerate((6, 7)):
                hT[m] = f1_correct(m, ps2[i])

            # ---- wx2 = Ws . x2sum from the Pool accumulator ----
            w16 = acts.tile([128, nrem], F16, name="w16", tag="w16")
            nc.scalar.copy(w16[:], wacc2[:])
            wxu = pstat("ps_wx2", nrem)
            nc.tensor.matmul(wxu, ones_col[:], w16[:], start=True, stop=True)
            wx2 = acts.tile([1, nrem], F32, name="wx2", tag="wx2")
            nc.vector.tensor_tensor(wx2[:], wxu, rstd2[:], ALU.mult)
            nc.vector.scalar_tensor_tensor(wx2[:], nmr2[:],
                                           consts[0:1, 0:1], wx2[:],
                                           ALU.mult, ALU.add)

            # ---- linear logit stats from hT: s1 = c2.g ; ws += w2s.g/256 --
            c2p = pstat("ps_c2", nrem)
            for m in range(8):
                nc.tensor.matmul(c2p, c2w_sb[:, m:m + 1], hT[m][:],
                                 start=(m == 0), stop=(m == 7))
            w2p = pstat("ps_w2s", nrem)
            for m in range(8):
                nc.tensor.matmul(w2p, c2w_sb[:, 8 + m:9 + m], hT[m][:],
                                 start=(m == 0), stop=(m == 7))
            s1part = acts.tile([1, 512], F32, name="s1part", tag="s1part")
            wspart = acts.tile([1, 512], F32, name="wspart", tag="wspart")
            nc.vector.memset(s1part[:], 1.0)
            nc.vector.memset(wspart[:], 0.0)
            nc.vector.tensor_copy(s1part[:, 0:nrem], c2p)
            nc.vector.tensor_scalar(wx2[:], wx2[:], 1.0 / NCORES, 0.0,
                                    ALU.mult, ALU.add)
            nc.vector.scalar_tensor_tensor(wspart[:, 0:nrem], w2p,
                                           1.0 / 256.0, wx2[:],
                                           ALU.mult, ALU.add)
            nc.sync.dma_start(statso_d.ap()[:, 0:512], s1part[:])
            nc.sync.dma_start(statso_d.ap()[:, 512:1024], wspart[:])

            # ---- materialize x2 in place for the f2 residual fold ----
            ln_apply(rem_ts, nrem, A2sb, B2sb, nc.vector,
                     dump=dbg.get("dbg_x2"))

            # ---- FFN f2: partial = x2/8 + Wf2_shard^T hT; RS/quarter ----
            arin4 = [dram.tile([128, 16 * nrem], F16, name=f"ar4i{q}",
                               tag=f"ar4i{q}") for q in range(2)]
            rs4 = [dram.tile([16, 16 * nrem], F16, name=f"rs4{q}",
                             tag=f"rs4{q}") for q in range(2)]
            for q in range(4):
                for sub in range(2):
                    wt = wchunk(f"w_f2_{q}_{sub}", 8 * 512)
                    nc.sync.dma_start(
                        wt[:], wd["f2"].ap()[:, 4096 * (2 * q + sub):
                                             4096 * (2 * q + sub + 1)])
                    ps = [pp(f"ps_f2_{q}_{sub}_{mi}", nrem)
                          for mi in range(4)]
                    for k in range(8):
                        for mi in range(4):
                            nc.tensor.matmul(
                                ps[mi],
                                wt[:, 512 * k + 128 * mi:
                                   512 * k + 128 * (mi + 1)],
                                hT[k][:],
                                start=(k == 0), stop=(k == 7))
                    st = acts.tile([128, 4 * nrem], F16,
                                   name=f"st_f2_{q}_{sub}",
                                   tag="stage", bufs=3,
                                   padded_shape=[128, 4 * 512])
                    for mi in range(4):
                        mm = 4 * sub + mi
                        m = 8 * q + mm
                        nc.vector.scalar_tensor_tensor(
                            st[:, mi * nrem:(mi + 1) * nrem],
                            xs(rem_ts, nrem, m), 1.0 / NCORES, ps[mi],
                            ALU.mult, ALU.add)
                    off = ((8 * (q % 2)) + 4 * sub) * nrem
                    nc.sync.dma_start(
                        arin4[q // 2][:, off:off + 4 * nrem], st[:])
                if q % 2 == 1:
                    nc.gpsimd.collective_compute(
                        "ReduceScatter", ALU.add, replica_groups=replica,
                        ins=[arin4[q // 2].opt()],
                        outs=[rs4[q // 2].opt()])

            # ---- s2 from the scattered summed features, chased/half ----
            CW = 4 * nrem  # columns per rs4 read chunk (4 chunks/half)
            s2p = pstat("ps_rs2", nrem)
            for cch in range(8):
                q, hcol = cch // 4, (cch % 4) * CW
                bt = acts.tile([16, CW], F16, name=f"rsb_{cch}", tag="rsb",
                               bufs=2, padded_shape=[16, 4 * 512])
                nc.sync.dma_start(bt[:], rs4[q][:, hcol:hcol + CW])
                sq = acts.tile([16, CW], F16, name=f"rssq_{cch}", tag="rssq",
                               bufs=2, padded_shape=[16, 4 * 512])
                nc.scalar.square(sq[:], bt[:])
                for s in range(4):
                    k = cch * 4 + s
                    nc.tensor.matmul(s2p, ones_col[0:16, :],
                                     sq[:, s * nrem:(s + 1) * nrem],
                                     start=(k == 0), stop=(k == ET - 1))
            s2part = acts.tile([1, 512], F32, name="s2part", tag="s2part")
            nc.vector.memset(s2part[:], 1.0)
            nc.vector.tensor_copy(s2part[:, 0:nrem], s2p)
            nc.sync.dma_start(statso_d.ap()[:, 1024:1536], s2part[:])

    nc.compile()
    return nc


# ----------------------------------------------------------------------------
# host orchestration
# ----------------------------------------------------------------------------

def _packx(XT):
    """[E, L] fp32 -> [128, ET*L] fp16 feature-block pack."""
    L = XT.shape[1]
    return np.ascontiguousarray(
        XT.reshape(ET, 128, L).transpose(1, 0, 2).reshape(128, ET * L)
        .astype(np.float16))


def _prep_in_maps(vision_feature, text_embed, sel_idx, rem_idx,
                  Wqkv1, Wo1, Wqkv2, Wo2, Wqkvc, Woc, Wf1, Wf2, Ws):
    f16 = np.float16
    sel = vision_feature[sel_idx]
    rem = vision_feature[rem_idx]
    cat = np.concatenate([sel, text_embed], axis=0)

    remp = _packx(np.ascontiguousarray(rem.T))
    catp = _packx(np.ascontiguousarray(cat.T))
    ws_pack = np.ascontiguousarray(Ws[0].reshape(ET, 128).T.astype(f16))
    consts = np.broadcast_to(
        np.array([[np.float64(Ws.astype(np.float64).sum()), 0.0]],
                 np.float32), (128, 2)).copy()

    in_maps = []
    for c in range(NCORES):
        hs = slice(DLOC * c, DLOC * (c + 1))
        fs = slice(FLOC * c, FLOC * (c + 1))
        m = {"remp": remp, "catp": catp, "wsp": ws_pack, "consts": consts}
        for l, Wqkv, Wo in (("1", Wqkv1, Wo1), ("2", Wqkv2, Wo2),
                            ("c", Wqkvc, Woc)):
            Wq, Wk, Wv = Wqkv[:E], Wqkv[E:2 * E], Wqkv[2 * E:]
            for nm, W in (("q", Wq), ("k", Wk), ("v", Wv)):
                A = W[hs].T  # [E, DLOC]
                m[f"w{nm}{l}"] = np.ascontiguousarray(
                    A.reshape(ET, 128, DLOC).transpose(1, 0, 2)
                    .reshape(128, ET * DLOC).astype(f16))
            WoT = Wo[:, hs].T  # [DLOC, E]
            m[f"wo{l}"] = np.ascontiguousarray(
                WoT.reshape(4, 128, 4, 1024).transpose(1, 2, 0, 3)
                .reshape(128, 4 * E).astype(f16))
        # (Wq_c^T 1) for the commuted-LN fixup of qc
        m["wq1r"] = np.ascontiguousarray(
            Wqkvc[:E][hs].astype(np.float64).sum(axis=1).reshape(1, DLOC)
            .astype(f16))
        A = Wf1[fs].T  # [E, FLOC]
        m["wf1a"] = np.ascontiguousarray(
            A[:, 0:768].reshape(ET, 128, 768).transpose(1, 0, 2)
            .reshape(128, ET * 768).astype(f16))
        m["wf1b"] = np.ascontiguousarray(
            A[:, 768:1024].reshape(ET, 128, 256).transpose(1, 0, 2)
            .reshape(128, ET * 256).astype(f16))
        W2T = Wf2[:, fs].T  # [FLOC, E]
        m["wf2"] = np.ascontiguousarray(
            W2T.reshape(8, 128, 8, 512).transpose(1, 2, 0, 3)
            .reshape(128, 8 * E).astype(f16))
        c2 = Wf2[:, fs].astype(np.float64).sum(axis=0)  # [FLOC]
        w2s = 256.0 * (Ws[0].astype(np.float64) @ Wf2[:, fs].astype(np.float64))
        c2w = np.concatenate([c2.reshape(8, 128).T, w2s.reshape(8, 128).T],
                             axis=1)  # [128, 16]
        m["c2w"] = np.ascontiguousarray(c2w.astype(f16))
        m["f1rs"] = np.ascontiguousarray(
            Wf1[fs].astype(np.float64).sum(axis=1).reshape(1, FLOC)
            .astype(f16))
        in_maps.append(m)
    return in_maps


def run_device(in_maps, ncat_real, nrem_real, dumps=False, trace=False):
    from concourse.bass_utils import run_bass_kernel_spmd

    key = (ncat_real, nrem_real, dumps)
    if key not in _CACHE:
        _CACHE[key] = _build_device(ncat_real, nrem_real, dumps=dumps)
    nc = _CACHE[key]
    return run_bass_kernel_spmd(nc, in_maps, list(range(NCORES)), trace=trace)


def _kernel_impl(inputs, debug=False, trace=False):
    vision_feature = np.asarray(inputs["vision_feature"], np.float32)
    text_embed = np.asarray(inputs["text_embed"], np.float32)
    attention_mask = np.asarray(inputs["attention_mask"])

    biases_zero = all(
        not np.any(np.asarray(inputs[b]))
        for b in ("bqkv1", "bo1", "bqkv2", "bo2", "bqkvc", "boc",
                  "bf1", "bf2", "bs"))
    if (not bool(attention_mask.all())) or (not biases_zero):
        return (_reference_np(**{k: np.asarray(v) for k, v in inputs.items()}),
                None)

    t, sel_idx, rem_idx = _score_partition(vision_feature, text_embed,
                                           attention_mask)
    ncat_real = t + text_embed.shape[0]
    nrem_real = vision_feature.shape[0] - t
    kk = int(t * EXPAND)

    in_maps = _prep_in_maps(
        vision_feature, text_embed, sel_idx, rem_idx,
        np.asarray(inputs["Wqkv1"], np.float32),
        np.asarray(inputs["Wo1"], np.float32),
        np.asarray(inputs["Wqkv2"], np.float32),
        np.asarray(inputs["Wo2"], np.float32),
        np.asarray(inputs["Wqkvc"], np.float32),
        np.asarray(inputs["Woc"], np.float32),
        np.asarray(inputs["Wf1"], np.float32),
        np.asarray(inputs["Wf2"], np.float32),
        np.asarray(inputs["Ws"], np.float32))
    res = run_device(in_maps, ncat_real, nrem_real, dumps=debug, trace=trace)
    g = np.zeros(1536, np.float64)
    for c in range(NCORES):
        g += res.results[c]["statso"][0].astype(np.float64)
    s1 = g[0:nrem_real]
    ws = g[512:512 + nrem_real]
    s2 = g[1024:1024 + nrem_real]
    mean = s1 / E
    var = s2 / E - mean * mean
    rstd = 1.0 / np.sqrt(var + 1e-5)
    wsum = float(np.asarray(inputs["Ws"], np.float64).sum())
    logits = rstd * ws - mean * rstd * wsum
    es = (1.0 / (1.0 + np.exp(-logits.astype(np.float32))))
    ei = np.argsort(-es, kind="stable")[:kk]
    final = np.sort(np.concatenate([sel_idx, rem_idx[ei]]))
    return vision_feature[final], res


def kernel(**inputs):
    out, _ = _kernel_impl(inputs)
    return out
